# revision 67
# baseline (speedup 1.0000x reference)
"""GQA kernel for TRN2, 8-way tensor-parallel by KV head (v2).

Per core i: KV head i, Q heads 4i..4i+3. All matmuls bf16 (full PE rate at any
free size). Cost-model-driven design:
  - Coalesced DMAs: host pre-lays x^T as [128, 16, B*T] so each 512-col chunk
    loads in ONE descriptor-dense DMA (HWDGE hold is ~625ns per DMA).
  - Scores S^T = K Q^T per 128-key block, two heads side-by-side in one
    [128, 2, 512] PSUM duo tile; ONE exp per block over both heads via 3-D AP,
    diagonal blocks trimmed to the causally valid column window.
  - Causality: diagonal-first AV accumulation with subrange matmuls — invalid
    columns are never streamed, so no zero-memset and no wasted PE rows.
  - Denominator via ones-column in V^T (rides along in the AV matmul).
  - V projected directly in [t, d] orientation (x-chunk stationary), no PE
    transposes.
  - Deferred normalization: Y^T drained unnormalized per chunk; recip (DVE),
    partition-broadcast + multiply (Pool) in chunk-wide ops.
  - Out projection per chunk from normalized Y^T; PSUM staged to SBUF bf16
    (DVE/Act alternating) and DMA'd; host sums the 8 partial outputs.
  - Manual interleave: proj/out-proj matmuls woven between attention blocks so
    the PE never idles during the Act-bound exp phase.
"""

import sys

for p in ("/opt/trn_rl_repo", "/root/.axon_site/_ro/trn_rl_repo"):
    if p not in sys.path:
        sys.path.insert(0, p)

import numpy as np
import ml_dtypes
from collections import deque
from contextlib import ExitStack

import concourse.bacc as bacc
import concourse.mybir as mybir
import concourse.tile as tile

F32 = mybir.dt.float32
BF16 = mybir.dt.bfloat16
BF16_NP = ml_dtypes.bfloat16
EXP = mybir.ActivationFunctionType.Exp

D = 64
ROPE_BASE = 10000.0
AC = 512  # t-chunk


def build_nc(C, T, B):
    CT = C // 128          # contraction tiles (16)
    NCH = T // AC          # chunks per batch (4)
    BT = B * T
    KPB = T // 128         # key blocks per batch (16)

    nc = bacc.Bacc("TRN2", target_bir_lowering=False, debug=False)

    xTr = nc.dram_tensor("xTr", [128, CT, BT], BF16, kind="ExternalInput")
    wqr = nc.dram_tensor("wqr", [128, CT, 256], BF16, kind="ExternalInput")
    wkr = nc.dram_tensor("wkr", [128, CT, 64], BF16, kind="ExternalInput")
    wvr = nc.dram_tensor("wvr", [128, CT, 64], BF16, kind="ExternalInput")
    wor = nc.dram_tensor("wor", [128, 2, C], BF16, kind="ExternalInput")
    rqc = nc.dram_tensor("rqc", [128, T], BF16, kind="ExternalInput")
    rqs = nc.dram_tensor("rqs", [128, T], BF16, kind="ExternalInput")
    rkc = nc.dram_tensor("rkc", [64, T], BF16, kind="ExternalInput")
    rks = nc.dram_tensor("rks", [64, T], BF16, kind="ExternalInput")
    tri2 = nc.dram_tensor("tri2", [128, 2, 128], BF16, kind="ExternalInput")
    out = nc.dram_tensor("out", [BT, C], BF16, kind="ExternalOutput")

    with tile.TileContext(nc) as tc, ExitStack() as ctx:
        # PSUM: pj 2 + ss 4 + yy 2 = 8 banks
        pj = ctx.enter_context(tc.tile_pool(name="pj", bufs=2, space="PSUM"))
        ss = ctx.enter_context(tc.tile_pool(name="ss", bufs=2, space="PSUM"))
        yy = ctx.enter_context(tc.tile_pool(name="yy", bufs=1, space="PSUM"))

        cst = ctx.enter_context(tc.tile_pool(name="cst", bufs=1))
        xcp = ctx.enter_context(tc.tile_pool(name="xcp", bufs=3))
        ktp = ctx.enter_context(tc.tile_pool(name="ktp", bufs=2))
        vpp = ctx.enter_context(tc.tile_pool(name="vpp", bufs=2))
        qsp = ctx.enter_context(tc.tile_pool(name="qsp", bufs=4))
        qtp = ctx.enter_context(tc.tile_pool(name="qtp", bufs=8))
        tmp = ctx.enter_context(tc.tile_pool(name="tmp", bufs=4))
        ppp = ctx.enter_context(tc.tile_pool(name="ppp", bufs=4))
        ytu = ctx.enter_context(tc.tile_pool(name="ytu", bufs=2))
        ytp = ctx.enter_context(tc.tile_pool(name="ytp", bufs=4))
        osp = ctx.enter_context(tc.tile_pool(name="osp", bufs=4))
        ncp = ctx.enter_context(tc.tile_pool(name="ncp", bufs=4))

        # ---- constants ----
        XW = cst.tile([128, CT, 256], BF16, tag="XW")
        KW = cst.tile([128, CT, 64], BF16, tag="KW")
        VW = cst.tile([128, CT, 64], BF16, tag="VW")
        OW = cst.tile([128, 2, C], BF16, tag="OW")
        RQC = cst.tile([128, T], BF16, tag="RQC")
        RQS = cst.tile([128, T], BF16, tag="RQS")
        RKC = cst.tile([64, T], BF16, tag="RKC")
        RKS = cst.tile([64, T], BF16, tag="RKS")
        TRI = cst.tile([128, 2, 128], BF16, tag="TRI")

        PROJ_ROPES = {}
        XC = {}   # (b, ch) -> x chunk tile [128, CT, AC]
        KT = {}   # b -> [64, T]
        VP = {}   # b -> [128, KPB, 65]
        QT = {}   # (ch%2, h) -> [64, AC]
        YT = {}   # (b, cl) -> [128, T]

        copy_flip = [0]

        def stage_copy(dst, src):
            """PSUM->SBUF staging, alternating DVE / Act."""
            copy_flip[0] ^= 1
            if copy_flip[0]:
                nc.vector.tensor_copy(dst, src)
            else:
                nc.scalar.copy(dst, src)

        def emit_xdma(b, ch):
            t = xcp.tile([128, CT, AC], BF16, tag="XC", name=f"XC{b}_{ch}")
            nc.sync.dma_start(t[:, :, :], xTr[:, :, b * T + ch * AC:b * T + (ch + 1) * AC])
            XC[(b, ch)] = t

        def fillers_proj(b, ch):
            """Closures projecting chunk (b, ch): K, V, Q0, Q1."""
            xc = XC[(b, ch)]
            tcol = ch * AC
            res = []

            pk_box, pv_box = [], []
            ks_box, qs_box = [], {}

            def k_mm(c0):
                if c0 == 0:
                    pk_box.append(pj.tile([64, AC], F32, tag="pj", name=f"PK{b}_{ch}"))
                pk = pk_box[0]
                for c in range(c0, c0 + 4):
                    nc.tensor.matmul(pk[:], KW[:, c, :], xc[:, c, :],
                                     start=(c == 0), stop=(c == CT - 1))

            def k_copy():
                ks = qsp.tile([64, AC], BF16, tag="KS", name=f"KS{b}_{ch}")
                nc.vector.tensor_copy(ks[:], pk_box[0][:])
                ks_box.append(ks)

            def k_rope():
                ks = ks_box[0]
                kt = KT[b]
                t1 = tmp.tile([64, AC], BF16, tag="kt1")
                t2 = tmp.tile([64, AC], BF16, tag="kt2")
                nc.vector.tensor_mul(t1[:], ks[:], RKC[:, tcol:tcol + AC])
                nc.vector.tensor_mul(t2[0:32, :], ks[32:64, :], RKS[32:64, tcol:tcol + AC])
                nc.vector.tensor_mul(t2[32:64, :], ks[0:32, :], RKS[0:32, tcol:tcol + AC])
                nc.vector.tensor_add(kt[:, tcol:tcol + AC], t1[:], t2[:])

            def v_mm(tb):
                if tb == 0:
                    pv_box.append(pj.tile([128, 4, 64], F32, tag="pj", name=f"PV{b}_{ch}"))
                pv = pv_box[0]
                for c in range(CT):
                    nc.tensor.matmul(pv[:, tb, :],
                                     xc[:, c, tb * 128:(tb + 1) * 128], VW[:, c, :],
                                     start=(c == 0), stop=(c == CT - 1))

            def v_stage():
                nc.vector.tensor_copy(VP[b][:, ch * 4:(ch + 1) * 4, 0:64], pv_box[0][:, :, :])

            def q_mm(c0, hp, pq_box):
                if c0 == 0:
                    pq_box.append(pj.tile([128, AC], F32, tag="pj", name=f"PQ{b}_{ch}_{hp}"))
                pq = pq_box[0]
                for c in range(c0, c0 + 4):
                    nc.tensor.matmul(pq[:], XW[:, c, hp * 128:(hp + 1) * 128], xc[:, c, :],
                                     start=(c == 0), stop=(c == CT - 1))

            def q_copy(hp, pq_box):
                qs = qsp.tile([128, AC], BF16, tag="QS", name=f"QS{b}_{ch}_{hp}")
                nc.vector.tensor_copy(qs[:], pq_box[0][:])
                qs_box[hp] = qs

            def q_rope(hp):
                qs = qs_box[hp]
                t1 = tmp.tile([128, AC], BF16, tag="qt1")
                t2 = tmp.tile([128, AC], BF16, tag="qt2")
                nc.vector.tensor_mul(t1[:], qs[:], RQC[:, tcol:tcol + AC])
                for b0 in (0, 64):
                    nc.vector.tensor_mul(t2[b0:b0 + 32, :], qs[b0 + 32:b0 + 64, :],
                                         RQS[b0 + 32:b0 + 64, tcol:tcol + AC])
                    nc.vector.tensor_mul(t2[b0 + 32:b0 + 64, :], qs[b0:b0 + 32, :],
                                         RQS[b0:b0 + 32, tcol:tcol + AC])
                for hl in range(2):
                    h = 2 * hp + hl
                    qt = qtp.tile([64, AC], BF16, tag="QT", name=f"QT{b}_{ch}_{h}")
                    nc.vector.tensor_add(qt[:], t1[hl * 64:hl * 64 + 64, :],
                                         t2[hl * 64:hl * 64 + 64, :])
                    QT[(ch % 2, h)] = qt

            # psum->sbuf copies right behind each chain (frees pj bufs fast);
            # long DVE rope bursts deferred to the end
            for c0 in range(0, CT, 4):
                res.append((875, lambda c0=c0: k_mm(c0)))
            res.append((10, k_copy))
            for tb in range(4):
                res.append((430, lambda tb=tb: v_mm(tb)))
            res.append((10, v_stage))
            pq_boxes = [[], []]
            for hp in range(2):
                for c0 in range(0, CT, 4):
                    res.append((875, lambda c0=c0, hp=hp: q_mm(c0, hp, pq_boxes[hp])))
                res.append((10, lambda hp=hp: q_copy(hp, pq_boxes[hp])))
            ropes = [(150, k_rope), (150, lambda: q_rope(0)),
                     (150, lambda: q_rope(1))]
            return res, ropes

        TAIL = [False]

        def fillers_outproj(b, ch):
            """Closures for out projection of chunk (b, ch) (needs YT cols)."""
            res = []

            def po_group(tt, co):
                trow = b * T + ch * AC + tt * 128
                po = pj.tile([128, 512], F32, tag="pj", name=f"PO{b}_{ch}_{tt}_{co}")
                for cl in range(2):
                    nc.tensor.matmul(po[:], YT[(b, cl)][:, ch * 4 * 128 + tt * 128:ch * 4 * 128 + (tt + 1) * 128],
                                     OW[:, cl, co * 512:(co + 1) * 512],
                                     start=(cl == 0), stop=(cl == 1))
                os_ = osp.tile([128, 512], BF16, tag="OS")
                stage_copy(os_[:], po[:])
                nc.sync.dma_start(out[trow:trow + 128, co * 512:(co + 1) * 512], os_[:])

            for tt in range(4):
                for co in range(C // 512):
                    res.append((440, lambda tt=tt, co=co: po_group(tt, co)))
            return res

        projq = deque()   # (cost_ns, closure) — must drain before next chunk
        sideq = deque()   # out-proj groups — drain lazily
        qcost = [0]       # total cost queued

        def push(q, items):
            q.extend(items)
            qcost[0] += sum(c for c, _ in items)

        reserve = [0]

        def head_cost():
            if projq:
                return projq[0][0]
            if sideq and qcost[0] > reserve[0]:
                return sideq[0][0]
            return None

        def fill(budget):
            while budget > 0:
                if projq:
                    c, f = projq.popleft()
                elif sideq and qcost[0] > reserve[0]:
                    c, f = sideq.popleft()
                else:
                    return
                f()
                qcost[0] -= c
                budget -= c

        def flush_proj():
            while projq:
                c, f = projq.popleft()
                qcost[0] -= c
                f()

        def flush_all():
            flush_proj()
            while sideq:
                c, f = sideq.popleft()
                qcost[0] -= c
                f()

        def attention_chunk(b, ch, budget, last_chunk):
            """Both head-pair passes of chunk (b, ch) as one pipelined stream."""
            kt = KT[b]
            vp = VP[b]
            kis = [("d", l) for l in range(4)] + [("f", k) for k in range(ch * 4)]
            n = len(kis)
            ytuc = ytu.tile([65, 4, AC], BF16, tag="YTU", name=f"YTU{b}_{ch}")
            yps = {}
            pend = deque()

            def emit_av(e):
                p2, f0, pr, idx, ki_ = e
                for j in range(2):
                    nc.tensor.matmul(yps[pr][:, j, f0:512], vp[:, ki_, :], p2[:, j, f0:512],
                                     start=(idx == 0), stop=(idx == n - 1))
                if idx == n - 1:
                    if last_chunk and pr == 1:
                        normalize_direct(b, ch, yps[pr], pr)
                    else:
                        normalize(b, ch, ytuc, yps[pr], pr)

            for pr in range(2):
                for i, (kind, v) in enumerate(kis):
                    ki = ch * 4 + v if kind == "d" else v
                    f0 = v * 128 if kind == "d" else 0
                    if i == 0:
                        yps[pr] = yy.tile([65, 2, 512], F32, tag="yy", name=f"Y{b}_{ch}_{pr}")
                    s2 = ss.tile([128, 2, 512], F32, tag="ss", name=f"S{b}_{ch}_{pr}_{i}")
                    for j in range(2):
                        h = 2 * pr + j
                        nc.tensor.matmul(s2[:, j, f0:512], kt[:, ki * 128:(ki + 1) * 128],
                                         QT[(ch % 2, h)][:, f0:512], start=True, stop=True)
                    p2 = ppp.tile([128, 2, 512], BF16, tag="P2")
                    nc.scalar.activation(p2[:, :, f0:512], s2[:, :, f0:512], EXP)
                    if kind == "d":
                        nc.vector.tensor_mul(p2[:, :, f0:f0 + 128], p2[:, :, f0:f0 + 128], TRI[:, :, :])
                    pend.append((p2, f0, pr, i, ki))
                    if len(pend) > 2:
                        emit_av(pend.popleft())
                    fill(800)
            while pend:
                emit_av(pend.popleft())

        def normalize(b, ch, ytuc, yps, pr):
            """Normalize the two heads of pass pr into YT.

            Recips read the denominator rows straight from PSUM (ahead of the
            Y drain); the two multiplies run on different engines in parallel.
            """
            tcol = ch * AC
            nc.vector.tensor_copy(ytuc[:, 2 * pr:2 * pr + 2, :], yps[:, :, :])
            rcs = []
            for j in range(2):
                rc = ncp.tile([1, AC], F32, tag="RC")
                nc.vector.reciprocal(rc[0:1, :], ytuc[64:65, 2 * pr + j, :])
                rcs.append(rc)
            bcs = []
            for j in range(2):
                bc = ncp.tile([64, AC], F32, tag="BC")
                nc.gpsimd.partition_broadcast(bc[:], rcs[j][0:1, :])
                bcs.append(bc)
            for j in range(2):
                h = 2 * pr + j
                cl, r0 = h // 2, (h % 2) * 64
                nc.gpsimd.tensor_mul(YT[(b, cl)][r0:r0 + 64, tcol:tcol + AC],
                                     ytuc[0:64, h, :], bcs[j][:])

        def normalize_direct(b, ch, yps, pr):
            """Tail fast path: normalize straight from PSUM (no drain copy),
            DVE muls, h-pipelined recip/bcast."""
            tcol = ch * AC
            rcs, bcs = [], []
            for j in range(2):
                rc = ncp.tile([1, AC], F32, tag="RC")
                nc.vector.reciprocal(rc[0:1, :], yps[64:65, j, :])
                rcs.append(rc)
            for j in range(2):
                bc = ncp.tile([64, AC], F32, tag="BC")
                nc.gpsimd.partition_broadcast(bc[:], rcs[j][0:1, :])
                bcs.append(bc)
            for j in range(2):
                h = 2 * pr + j
                cl, r0 = h // 2, (h % 2) * 64
                nc.vector.tensor_mul(YT[(b, cl)][r0:r0 + 64, tcol:tcol + AC],
                                     yps[0:64, j, :], bcs[j][:])

        # ---- PE warmup: ramp the p-state while the first DMAs land ----
        WRM = cst.tile([128, 512], BF16, tag="WRM")
        nc.vector.memset(WRM[:], 0.0)
        pwarm = pj.tile([128, 512], F32, tag="pj", name="PWARM")
        for _ in range(18):
            nc.tensor.matmul(pwarm[:], WRM[:, 0:128], WRM[:], start=True, stop=True)

        # ---- preamble DMAs (need-ordered; x chunk 0 split so K proj can
        #      start as soon as its weights + first c-rows arrive) ----
        x00 = xcp.tile([128, CT, AC], BF16, tag="XC", name="XC0_0")
        XC[(0, 0)] = x00
        nc.sync.dma_start(KW[:, 0:8, :], wkr[:, 0:8, :])
        nc.sync.dma_start(x00[:, 0:4, :], xTr[:, 0:4, 0:AC])
        nc.sync.dma_start(KW[:, 8:CT, :], wkr[:, 8:CT, :])
        for q in range(1, 4):
            nc.sync.dma_start(x00[:, 4 * q:4 * (q + 1), :],
                              xTr[:, 4 * q:4 * (q + 1), 0:AC])
        nc.sync.dma_start(RKC[:], rkc[:])
        nc.sync.dma_start(RKS[:], rks[:])
        nc.sync.dma_start(VW[:, :, :], wvr[:, :, :])
        nc.sync.dma_start(XW[:, :, :], wqr[:, :, :])
        nc.sync.dma_start(RQC[:], rqc[:])
        nc.sync.dma_start(RQS[:], rqs[:])
        nc.sync.dma_start(TRI[:, :, :], tri2[:, :, :])
        nc.sync.dma_start(OW[:, :, :], wor[:, :, :])
        emit_xdma(0, 1)

        for b in range(B):
            KT[b] = ktp.tile([64, T], BF16, tag="KT", name=f"KT{b}")
            VP[b] = vpp.tile([128, KPB, 65], BF16, tag="VP", name=f"VP{b}")
            nc.vector.memset(VP[b][:, :, 64:65], 1.0)
            for cl in range(2):
                YT[(b, cl)] = ytp.tile([128, T], BF16, tag="YT", name=f"YT{b}_{cl}")

        ca, ra = fillers_proj(0, 0)
        for _, c in ca + ra:
            c()

        def succ(b, ch, k):
            t = b * NCH + ch + k
            return (t // NCH, t % NCH) if t < B * NCH else None

        pushed = set()
        for b in range(B):
            for ch in range(NCH):
                flush_proj()  # this chunk's proj must be complete
                nxt = succ(b, ch, 2)
                if nxt:
                    emit_xdma(*nxt)
                # queue fillers: chains for chunk+1 (if new) + its ropes,
                # then chains of chunk+2 (QT-parity-safe extra supply)
                nxt = succ(b, ch, 1)
                if nxt:
                    if nxt not in pushed:
                        ca, ra = fillers_proj(*nxt)
                        push(projq, ca)
                        pushed.add(nxt)
                        PROJ_ROPES[nxt] = ra
                    push(projq, PROJ_ROPES.pop(nxt))
                nxt = succ(b, ch, 2)
                if ch >= 2 and nxt and nxt not in pushed:
                    ca, ra = fillers_proj(*nxt)
                    push(projq, ca)
                    pushed.add(nxt)
                    PROJ_ROPES[nxt] = ra
                last = b == B - 1 and ch == NCH - 1
                reserve[0] = 6500 if last else 0
                attention_chunk(b, ch, 800, last)
                push(sideq, fillers_outproj(b, ch))
        TAIL[0] = True
        flush_all()

    nc.compile()
    return nc


def rope_tables(T, scale):
    inv = 1.0 / (ROPE_BASE ** (np.arange(0, D, 2, dtype=np.float32) / D))
    t = np.arange(T, dtype=np.float32)
    freqs = np.outer(t, inv)
    emb = np.concatenate([freqs, freqs], -1)
    cos = np.cos(emb).T.astype(np.float32) * scale
    sin = np.sin(emb).T.astype(np.float32) * scale
    sinX = np.empty((64, T), np.float32)
    sinX[0:32] = sin[32:64]
    sinX[32:64] = -sin[0:32]
    return np.ascontiguousarray(cos), np.ascontiguousarray(sinX)


def _pk(a, nblk):
    """[nblk*128, F] -> [128, nblk, F] contiguous bf16."""
    n, f = a.shape
    return np.ascontiguousarray(
        a.reshape(nblk, 128, f).transpose(1, 0, 2)).astype(BF16_NP)


def make_inputs(x, Wq, Wk, Wv, Wo):
    B, T, C = x.shape
    CT = C // 128
    xT = np.ascontiguousarray(x.reshape(B * T, C).T)
    qc, qs = rope_tables(T, 1.0 / np.sqrt(D).astype(np.float32))
    kc, ks = rope_tables(T, 1.0)
    tri = np.triu(np.ones((128, 128), np.float32))
    common = {
        "xTr": _pk(xT, CT),
        "rqc": np.concatenate([qc, qc], 0).astype(BF16_NP),
        "rqs": np.concatenate([qs, qs], 0).astype(BF16_NP),
        "rkc": kc.astype(BF16_NP),
        "rks": ks.astype(BF16_NP),
        "tri2": np.ascontiguousarray(
            np.stack([tri, tri], 1)).astype(BF16_NP),
    }
    in_maps = []
    for i in range(8):
        m = dict(common)
        m["wqr"] = _pk(np.ascontiguousarray(Wq[:, i * 256:(i + 1) * 256]), CT)
        m["wkr"] = _pk(np.ascontiguousarray(Wk[:, i * 64:(i + 1) * 64]), CT)
        m["wvr"] = _pk(np.ascontiguousarray(Wv[:, i * 64:(i + 1) * 64]), CT)
        m["wor"] = _pk(np.ascontiguousarray(Wo[i * 256:(i + 1) * 256, :]), 2)
        in_maps.append(m)
    return in_maps


_NC_CACHE = {}


def _get_nc(C, T, B):
    key = (C, T, B)
    if key not in _NC_CACHE:
        _NC_CACHE[key] = build_nc(C, T, B)
    return _NC_CACHE[key]


def run(x, Wq, Wk, Wv, Wo, trace=False):
    from concourse.bass_utils import run_bass_kernel_spmd

    B, T, C = x.shape
    nc = _get_nc(C, T, B)
    in_maps = make_inputs(x, Wq, Wk, Wv, Wo)
    for attempt in range(3):
        try:
            res = run_bass_kernel_spmd(nc, in_maps, list(range(8)), trace=trace)
        except (ImportError, ModuleNotFoundError):
            res = run_bass_kernel_spmd(nc, in_maps, list(range(8)), trace=False)
        acc = res.results[0]["out"].astype(np.float32)
        for i in range(1, 8):
            acc = acc + res.results[i]["out"].astype(np.float32)
        if np.isfinite(acc).all():
            break
    return acc.reshape(B, T, C), res


def kernel(x, Wq, Wk, Wv, Wo):
    out, _ = run(x, Wq, Wk, Wv, Wo, trace=False)
    return out


# revision 68
# speedup vs baseline: 1.0061x; 1.0061x over previous
"""GQA kernel for TRN2, 8-way tensor-parallel by KV head (v2).

Per core i: KV head i, Q heads 4i..4i+3. All matmuls bf16 (full PE rate at any
free size). Cost-model-driven design:
  - Coalesced DMAs: host pre-lays x^T as [128, 16, B*T] so each 512-col chunk
    loads in ONE descriptor-dense DMA (HWDGE hold is ~625ns per DMA).
  - Scores S^T = K Q^T per 128-key block, two heads side-by-side in one
    [128, 2, 512] PSUM duo tile; ONE exp per block over both heads via 3-D AP,
    diagonal blocks trimmed to the causally valid column window.
  - Causality: diagonal-first AV accumulation with subrange matmuls — invalid
    columns are never streamed, so no zero-memset and no wasted PE rows.
  - Denominator via ones-column in V^T (rides along in the AV matmul).
  - V projected directly in [t, d] orientation (x-chunk stationary), no PE
    transposes.
  - Deferred normalization: Y^T drained unnormalized per chunk; recip (DVE),
    partition-broadcast + multiply (Pool) in chunk-wide ops.
  - Out projection per chunk from normalized Y^T; PSUM staged to SBUF bf16
    (DVE/Act alternating) and DMA'd; host sums the 8 partial outputs.
  - Manual interleave: proj/out-proj matmuls woven between attention blocks so
    the PE never idles during the Act-bound exp phase.
"""

import sys

for p in ("/opt/trn_rl_repo", "/root/.axon_site/_ro/trn_rl_repo"):
    if p not in sys.path:
        sys.path.insert(0, p)

import numpy as np
import ml_dtypes
from collections import deque
from contextlib import ExitStack

import concourse.bacc as bacc
import concourse.mybir as mybir
import concourse.tile as tile

F32 = mybir.dt.float32
BF16 = mybir.dt.bfloat16
BF16_NP = ml_dtypes.bfloat16
EXP = mybir.ActivationFunctionType.Exp

D = 64
ROPE_BASE = 10000.0
AC = 512  # t-chunk


def build_nc(C, T, B):
    CT = C // 128          # contraction tiles (16)
    NCH = T // AC          # chunks per batch (4)
    BT = B * T
    KPB = T // 128         # key blocks per batch (16)

    nc = bacc.Bacc("TRN2", target_bir_lowering=False, debug=False)

    xTr = nc.dram_tensor("xTr", [128, CT, BT], BF16, kind="ExternalInput")
    wqr = nc.dram_tensor("wqr", [128, CT, 256], BF16, kind="ExternalInput")
    wkr = nc.dram_tensor("wkr", [128, CT, 64], BF16, kind="ExternalInput")
    wvr = nc.dram_tensor("wvr", [128, CT, 64], BF16, kind="ExternalInput")
    wor = nc.dram_tensor("wor", [128, 2, C], BF16, kind="ExternalInput")
    rqc = nc.dram_tensor("rqc", [128, T], BF16, kind="ExternalInput")
    rqs = nc.dram_tensor("rqs", [128, T], BF16, kind="ExternalInput")
    rkc = nc.dram_tensor("rkc", [64, T], BF16, kind="ExternalInput")
    rks = nc.dram_tensor("rks", [64, T], BF16, kind="ExternalInput")
    tri2 = nc.dram_tensor("tri2", [128, 2, 128], BF16, kind="ExternalInput")
    out = nc.dram_tensor("out", [BT, C], BF16, kind="ExternalOutput")

    with tile.TileContext(nc) as tc, ExitStack() as ctx:
        # PSUM: pj 2 + ss 4 + yy 2 = 8 banks
        pj = ctx.enter_context(tc.tile_pool(name="pj", bufs=2, space="PSUM"))
        ss = ctx.enter_context(tc.tile_pool(name="ss", bufs=2, space="PSUM"))
        yy = ctx.enter_context(tc.tile_pool(name="yy", bufs=1, space="PSUM"))

        cst = ctx.enter_context(tc.tile_pool(name="cst", bufs=1))
        xcp = ctx.enter_context(tc.tile_pool(name="xcp", bufs=3))
        ktp = ctx.enter_context(tc.tile_pool(name="ktp", bufs=2))
        vpp = ctx.enter_context(tc.tile_pool(name="vpp", bufs=2))
        qsp = ctx.enter_context(tc.tile_pool(name="qsp", bufs=4))
        qtp = ctx.enter_context(tc.tile_pool(name="qtp", bufs=8))
        tmp = ctx.enter_context(tc.tile_pool(name="tmp", bufs=4))
        ppp = ctx.enter_context(tc.tile_pool(name="ppp", bufs=4))
        ytu = ctx.enter_context(tc.tile_pool(name="ytu", bufs=2))
        ytp = ctx.enter_context(tc.tile_pool(name="ytp", bufs=4))
        osp = ctx.enter_context(tc.tile_pool(name="osp", bufs=4))
        ncp = ctx.enter_context(tc.tile_pool(name="ncp", bufs=4))

        # ---- constants ----
        XW = cst.tile([128, CT, 256], BF16, tag="XW")
        KW = cst.tile([128, CT, 64], BF16, tag="KW")
        VW = cst.tile([128, CT, 64], BF16, tag="VW")
        OW = cst.tile([128, 2, C], BF16, tag="OW")
        RQC = cst.tile([128, T], BF16, tag="RQC")
        RQS = cst.tile([128, T], BF16, tag="RQS")
        RKC = cst.tile([64, T], BF16, tag="RKC")
        RKS = cst.tile([64, T], BF16, tag="RKS")
        TRI = cst.tile([128, 2, 128], BF16, tag="TRI")

        PROJ_ROPES = {}
        XC = {}   # (b, ch) -> x chunk tile [128, CT, AC]
        KT = {}   # b -> [64, T]
        VP = {}   # b -> [128, KPB, 65]
        QT = {}   # (ch%2, h) -> [64, AC]
        YT = {}   # (b, cl) -> [128, T]

        copy_flip = [0]

        def stage_copy(dst, src):
            """PSUM->SBUF staging, alternating DVE / Act."""
            copy_flip[0] ^= 1
            if copy_flip[0]:
                nc.vector.tensor_copy(dst, src)
            else:
                nc.scalar.copy(dst, src)

        def emit_xdma(b, ch):
            t = xcp.tile([128, CT, AC], BF16, tag="XC", name=f"XC{b}_{ch}")
            nc.sync.dma_start(t[:, :, :], xTr[:, :, b * T + ch * AC:b * T + (ch + 1) * AC])
            XC[(b, ch)] = t

        def fillers_proj(b, ch):
            """Closures projecting chunk (b, ch): K, V, Q0, Q1."""
            xc = XC[(b, ch)]
            tcol = ch * AC
            res = []

            pk_box, pv_box = [], []
            ks_box, qs_box = [], {}

            def k_mm(c0):
                if c0 == 0:
                    pk_box.append(pj.tile([64, AC], F32, tag="pj", name=f"PK{b}_{ch}"))
                pk = pk_box[0]
                for c in range(c0, c0 + 4):
                    nc.tensor.matmul(pk[:], KW[:, c, :], xc[:, c, :],
                                     start=(c == 0), stop=(c == CT - 1))

            def k_copy():
                ks = qsp.tile([64, AC], BF16, tag="KS", name=f"KS{b}_{ch}")
                nc.vector.tensor_copy(ks[:], pk_box[0][:])
                ks_box.append(ks)

            def k_rope():
                ks = ks_box[0]
                kt = KT[b]
                t1 = tmp.tile([64, AC], BF16, tag="kt1")
                t2 = tmp.tile([64, AC], BF16, tag="kt2")
                nc.vector.tensor_mul(t1[:], ks[:], RKC[:, tcol:tcol + AC])
                nc.vector.tensor_mul(t2[0:32, :], ks[32:64, :], RKS[32:64, tcol:tcol + AC])
                nc.vector.tensor_mul(t2[32:64, :], ks[0:32, :], RKS[0:32, tcol:tcol + AC])
                nc.vector.tensor_add(kt[:, tcol:tcol + AC], t1[:], t2[:])

            def v_mm(tb):
                if tb == 0:
                    pv_box.append(pj.tile([128, 4, 64], F32, tag="pj", name=f"PV{b}_{ch}"))
                pv = pv_box[0]
                for c in range(CT):
                    nc.tensor.matmul(pv[:, tb, :],
                                     xc[:, c, tb * 128:(tb + 1) * 128], VW[:, c, :],
                                     start=(c == 0), stop=(c == CT - 1))

            def v_stage():
                nc.vector.tensor_copy(VP[b][:, ch * 4:(ch + 1) * 4, 0:64], pv_box[0][:, :, :])

            def q_mm(c0, hp, pq_box):
                if c0 == 0:
                    pq_box.append(pj.tile([128, AC], F32, tag="pj", name=f"PQ{b}_{ch}_{hp}"))
                pq = pq_box[0]
                for c in range(c0, c0 + 4):
                    nc.tensor.matmul(pq[:], XW[:, c, hp * 128:(hp + 1) * 128], xc[:, c, :],
                                     start=(c == 0), stop=(c == CT - 1))

            def q_copy(hp, pq_box):
                qs = qsp.tile([128, AC], BF16, tag="QS", name=f"QS{b}_{ch}_{hp}")
                nc.vector.tensor_copy(qs[:], pq_box[0][:])
                qs_box[hp] = qs

            def q_rope(hp):
                qs = qs_box[hp]
                t1 = tmp.tile([128, AC], BF16, tag="qt1")
                t2 = tmp.tile([128, AC], BF16, tag="qt2")
                nc.vector.tensor_mul(t1[:], qs[:], RQC[:, tcol:tcol + AC])
                for b0 in (0, 64):
                    nc.vector.tensor_mul(t2[b0:b0 + 32, :], qs[b0 + 32:b0 + 64, :],
                                         RQS[b0 + 32:b0 + 64, tcol:tcol + AC])
                    nc.vector.tensor_mul(t2[b0 + 32:b0 + 64, :], qs[b0:b0 + 32, :],
                                         RQS[b0:b0 + 32, tcol:tcol + AC])
                for hl in range(2):
                    h = 2 * hp + hl
                    qt = qtp.tile([64, AC], BF16, tag="QT", name=f"QT{b}_{ch}_{h}")
                    nc.vector.tensor_add(qt[:], t1[hl * 64:hl * 64 + 64, :],
                                         t2[hl * 64:hl * 64 + 64, :])
                    QT[(ch % 2, h)] = qt

            # psum->sbuf copies right behind each chain (frees pj bufs fast);
            # long DVE rope bursts deferred to the end
            for c0 in range(0, CT, 4):
                res.append((875, lambda c0=c0: k_mm(c0)))
            res.append((10, k_copy))
            for tb in range(4):
                res.append((430, lambda tb=tb: v_mm(tb)))
            res.append((10, v_stage))
            pq_boxes = [[], []]
            for hp in range(2):
                for c0 in range(0, CT, 4):
                    res.append((875, lambda c0=c0, hp=hp: q_mm(c0, hp, pq_boxes[hp])))
                res.append((10, lambda hp=hp: q_copy(hp, pq_boxes[hp])))
            ropes = [(150, k_rope), (150, lambda: q_rope(0)),
                     (150, lambda: q_rope(1))]
            return res, ropes

        TAIL = [False]

        def fillers_outproj(b, ch):
            """Closures for out projection of chunk (b, ch) (needs YT cols)."""
            res = []

            def po_group(tt, co):
                trow = b * T + ch * AC + tt * 128
                po = pj.tile([128, 512], F32, tag="pj", name=f"PO{b}_{ch}_{tt}_{co}")
                for cl in range(2):
                    nc.tensor.matmul(po[:], YT[(b, cl)][:, ch * 4 * 128 + tt * 128:ch * 4 * 128 + (tt + 1) * 128],
                                     OW[:, cl, co * 512:(co + 1) * 512],
                                     start=(cl == 0), stop=(cl == 1))
                os_ = osp.tile([128, 512], BF16, tag="OS")
                stage_copy(os_[:], po[:])
                nc.sync.dma_start(out[trow:trow + 128, co * 512:(co + 1) * 512], os_[:])

            for tt in range(4):
                for co in range(C // 512):
                    res.append((440, lambda tt=tt, co=co: po_group(tt, co)))
            return res

        projq = deque()   # (cost_ns, closure) — must drain before next chunk
        sideq = deque()   # out-proj groups — drain lazily
        qcost = [0]       # total cost queued

        def push(q, items):
            q.extend(items)
            qcost[0] += sum(c for c, _ in items)

        reserve = [0]

        def head_cost():
            if projq:
                return projq[0][0]
            if sideq and qcost[0] > reserve[0]:
                return sideq[0][0]
            return None

        def fill(budget):
            while budget > 0:
                if projq:
                    c, f = projq.popleft()
                elif sideq and qcost[0] > reserve[0]:
                    c, f = sideq.popleft()
                else:
                    return
                f()
                qcost[0] -= c
                budget -= c

        def flush_proj():
            while projq:
                c, f = projq.popleft()
                qcost[0] -= c
                f()

        def flush_all():
            flush_proj()
            while sideq:
                c, f = sideq.popleft()
                qcost[0] -= c
                f()

        def attention_chunk(b, ch, budget, last_chunk):
            """Both head-pair passes of chunk (b, ch) as one pipelined stream."""
            kt = KT[b]
            vp = VP[b]
            kis = [("d", l) for l in range(4)] + [("f", k) for k in range(ch * 4)]
            n = len(kis)
            ytuc = ytu.tile([65, 4, AC], BF16, tag="YTU", name=f"YTU{b}_{ch}")
            yps = {}
            pend = deque()

            def emit_av(e):
                p2, f0, pr, idx, ki_ = e
                for j in range(2):
                    nc.tensor.matmul(yps[pr][:, j, f0:512], vp[:, ki_, :], p2[:, j, f0:512],
                                     start=(idx == 0), stop=(idx == n - 1))
                if idx == n - 1:
                    if last_chunk and pr == 1:
                        normalize_direct(b, ch, yps[pr], pr)
                    else:
                        normalize(b, ch, ytuc, yps[pr], pr)

            for pr in range(2):
                for i, (kind, v) in enumerate(kis):
                    ki = ch * 4 + v if kind == "d" else v
                    f0 = v * 128 if kind == "d" else 0
                    if i == 0:
                        yps[pr] = yy.tile([65, 2, 512], F32, tag="yy", name=f"Y{b}_{ch}_{pr}")
                    s2 = ss.tile([128, 2, 512], F32, tag="ss", name=f"S{b}_{ch}_{pr}_{i}")
                    for j in range(2):
                        h = 2 * pr + j
                        nc.tensor.matmul(s2[:, j, f0:512], kt[:, ki * 128:(ki + 1) * 128],
                                         QT[(ch % 2, h)][:, f0:512], start=True, stop=True)
                    p2 = ppp.tile([128, 2, 512], BF16, tag="P2")
                    nc.scalar.activation(p2[:, :, f0:512], s2[:, :, f0:512], EXP)
                    if kind == "d":
                        nc.vector.tensor_mul(p2[:, :, f0:f0 + 128], p2[:, :, f0:f0 + 128], TRI[:, :, :])
                    pend.append((p2, f0, pr, i, ki))
                    if len(pend) > 2:
                        emit_av(pend.popleft())
                    fill(800)
            while pend:
                emit_av(pend.popleft())

        def normalize(b, ch, ytuc, yps, pr):
            """Normalize the two heads of pass pr into YT.

            Recips read the denominator rows straight from PSUM (ahead of the
            Y drain); the two multiplies run on different engines in parallel.
            """
            tcol = ch * AC
            nc.vector.tensor_copy(ytuc[:, 2 * pr:2 * pr + 2, :], yps[:, :, :])
            rcs = []
            for j in range(2):
                rc = ncp.tile([1, AC], F32, tag="RC")
                nc.vector.reciprocal(rc[0:1, :], ytuc[64:65, 2 * pr + j, :])
                rcs.append(rc)
            bcs = []
            for j in range(2):
                bc = ncp.tile([64, AC], F32, tag="BC")
                nc.gpsimd.partition_broadcast(bc[:], rcs[j][0:1, :])
                bcs.append(bc)
            for j in range(2):
                h = 2 * pr + j
                cl, r0 = h // 2, (h % 2) * 64
                nc.gpsimd.tensor_mul(YT[(b, cl)][r0:r0 + 64, tcol:tcol + AC],
                                     ytuc[0:64, h, :], bcs[j][:])

        def normalize_direct(b, ch, yps, pr):
            """Tail fast path: normalize straight from PSUM (no drain copy),
            DVE muls, h-pipelined recip/bcast."""
            tcol = ch * AC
            rcs, bcs = [], []
            for j in range(2):
                rc = ncp.tile([1, AC], F32, tag="RC")
                nc.vector.reciprocal(rc[0:1, :], yps[64:65, j, :])
                rcs.append(rc)
            for j in range(2):
                bc = ncp.tile([64, AC], F32, tag="BC")
                nc.gpsimd.partition_broadcast(bc[:], rcs[j][0:1, :])
                bcs.append(bc)
            for j in range(2):
                h = 2 * pr + j
                cl, r0 = h // 2, (h % 2) * 64
                nc.vector.tensor_mul(YT[(b, cl)][r0:r0 + 64, tcol:tcol + AC],
                                     yps[0:64, j, :], bcs[j][:])

        # ---- PE warmup: ramp the p-state while the first DMAs land ----
        WRM = cst.tile([128, 512], BF16, tag="WRM")
        nc.vector.memset(WRM[:], 0.0)
        pwarm = pj.tile([128, 512], F32, tag="pj", name="PWARM")
        for _ in range(18):
            nc.tensor.matmul(pwarm[:], WRM[:, 0:128], WRM[:], start=True, stop=True)

        # ---- preamble DMAs (need-ordered; x chunk 0 split so K proj can
        #      start as soon as its weights + first c-rows arrive) ----
        x00 = xcp.tile([128, CT, AC], BF16, tag="XC", name="XC0_0")
        XC[(0, 0)] = x00
        nc.sync.dma_start(KW[:, 0:8, :], wkr[:, 0:8, :])
        nc.sync.dma_start(x00[:, 0:4, :], xTr[:, 0:4, 0:AC])
        nc.sync.dma_start(KW[:, 8:CT, :], wkr[:, 8:CT, :])
        for q in range(1, 4):
            nc.sync.dma_start(x00[:, 4 * q:4 * (q + 1), :],
                              xTr[:, 4 * q:4 * (q + 1), 0:AC])
        nc.sync.dma_start(RKC[:], rkc[:])
        nc.sync.dma_start(RKS[:], rks[:])
        nc.sync.dma_start(VW[:, :, :], wvr[:, :, :])
        nc.sync.dma_start(XW[:, :, :], wqr[:, :, :])
        nc.sync.dma_start(RQC[:], rqc[:])
        nc.sync.dma_start(RQS[:], rqs[:])
        nc.sync.dma_start(TRI[:, :, :], tri2[:, :, :])
        nc.sync.dma_start(OW[:, :, :], wor[:, :, :])
        emit_xdma(0, 1)

        for b in range(B):
            KT[b] = ktp.tile([64, T], BF16, tag="KT", name=f"KT{b}")
            VP[b] = vpp.tile([128, KPB, 65], BF16, tag="VP", name=f"VP{b}")
            nc.vector.memset(VP[b][:, :, 64:65], 1.0)
            for cl in range(2):
                YT[(b, cl)] = ytp.tile([128, T], BF16, tag="YT", name=f"YT{b}_{cl}")

        ca, ra = fillers_proj(0, 0)
        for _, c in ca + ra:
            c()

        def succ(b, ch, k):
            t = b * NCH + ch + k
            return (t // NCH, t % NCH) if t < B * NCH else None

        pushed = set()
        for b in range(B):
            for ch in range(NCH):
                flush_proj()  # this chunk's proj must be complete
                nxt = succ(b, ch, 2)
                if nxt:
                    emit_xdma(*nxt)
                # queue fillers: chains for chunk+1 (if new) + its ropes,
                # then chains of chunk+2 (QT-parity-safe extra supply)
                nxt = succ(b, ch, 1)
                if nxt:
                    if nxt not in pushed:
                        ca, ra = fillers_proj(*nxt)
                        push(projq, ca)
                        pushed.add(nxt)
                        PROJ_ROPES[nxt] = ra
                    push(projq, PROJ_ROPES.pop(nxt))
                nxt = succ(b, ch, 2)
                if ch >= 2 and nxt and nxt not in pushed:
                    ca, ra = fillers_proj(*nxt)
                    push(projq, ca)
                    pushed.add(nxt)
                    PROJ_ROPES[nxt] = ra
                last = b == B - 1 and ch == NCH - 1
                reserve[0] = 4000 if last else 0
                attention_chunk(b, ch, 800, last)
                push(sideq, fillers_outproj(b, ch))
        TAIL[0] = True
        flush_all()

    nc.compile()
    return nc


def rope_tables(T, scale):
    inv = 1.0 / (ROPE_BASE ** (np.arange(0, D, 2, dtype=np.float32) / D))
    t = np.arange(T, dtype=np.float32)
    freqs = np.outer(t, inv)
    emb = np.concatenate([freqs, freqs], -1)
    cos = np.cos(emb).T.astype(np.float32) * scale
    sin = np.sin(emb).T.astype(np.float32) * scale
    sinX = np.empty((64, T), np.float32)
    sinX[0:32] = sin[32:64]
    sinX[32:64] = -sin[0:32]
    return np.ascontiguousarray(cos), np.ascontiguousarray(sinX)


def _pk(a, nblk):
    """[nblk*128, F] -> [128, nblk, F] contiguous bf16."""
    n, f = a.shape
    return np.ascontiguousarray(
        a.reshape(nblk, 128, f).transpose(1, 0, 2)).astype(BF16_NP)


def make_inputs(x, Wq, Wk, Wv, Wo):
    B, T, C = x.shape
    CT = C // 128
    xT = np.ascontiguousarray(x.reshape(B * T, C).T)
    qc, qs = rope_tables(T, 1.0 / np.sqrt(D).astype(np.float32))
    kc, ks = rope_tables(T, 1.0)
    tri = np.triu(np.ones((128, 128), np.float32))
    common = {
        "xTr": _pk(xT, CT),
        "rqc": np.concatenate([qc, qc], 0).astype(BF16_NP),
        "rqs": np.concatenate([qs, qs], 0).astype(BF16_NP),
        "rkc": kc.astype(BF16_NP),
        "rks": ks.astype(BF16_NP),
        "tri2": np.ascontiguousarray(
            np.stack([tri, tri], 1)).astype(BF16_NP),
    }
    in_maps = []
    for i in range(8):
        m = dict(common)
        m["wqr"] = _pk(np.ascontiguousarray(Wq[:, i * 256:(i + 1) * 256]), CT)
        m["wkr"] = _pk(np.ascontiguousarray(Wk[:, i * 64:(i + 1) * 64]), CT)
        m["wvr"] = _pk(np.ascontiguousarray(Wv[:, i * 64:(i + 1) * 64]), CT)
        m["wor"] = _pk(np.ascontiguousarray(Wo[i * 256:(i + 1) * 256, :]), 2)
        in_maps.append(m)
    return in_maps


_NC_CACHE = {}


def _get_nc(C, T, B):
    key = (C, T, B)
    if key not in _NC_CACHE:
        _NC_CACHE[key] = build_nc(C, T, B)
    return _NC_CACHE[key]


def run(x, Wq, Wk, Wv, Wo, trace=False):
    from concourse.bass_utils import run_bass_kernel_spmd

    B, T, C = x.shape
    nc = _get_nc(C, T, B)
    in_maps = make_inputs(x, Wq, Wk, Wv, Wo)
    for attempt in range(3):
        try:
            res = run_bass_kernel_spmd(nc, in_maps, list(range(8)), trace=trace)
        except (ImportError, ModuleNotFoundError):
            res = run_bass_kernel_spmd(nc, in_maps, list(range(8)), trace=False)
        acc = res.results[0]["out"].astype(np.float32)
        for i in range(1, 8):
            acc = acc + res.results[i]["out"].astype(np.float32)
        if np.isfinite(acc).all():
            break
    return acc.reshape(B, T, C), res


def kernel(x, Wq, Wk, Wv, Wo):
    out, _ = run(x, Wq, Wk, Wv, Wo, trace=False)
    return out


# revision 69
# speedup vs baseline: 1.0281x; 1.0219x over previous
"""GQA kernel for TRN2, 8-way tensor-parallel by KV head (v2).

Per core i: KV head i, Q heads 4i..4i+3. All matmuls bf16 (full PE rate at any
free size). Cost-model-driven design:
  - Coalesced DMAs: host pre-lays x^T as [128, 16, B*T] so each 512-col chunk
    loads in ONE descriptor-dense DMA (HWDGE hold is ~625ns per DMA).
  - Scores S^T = K Q^T per 128-key block, two heads side-by-side in one
    [128, 2, 512] PSUM duo tile; ONE exp per block over both heads via 3-D AP,
    diagonal blocks trimmed to the causally valid column window.
  - Causality: diagonal-first AV accumulation with subrange matmuls — invalid
    columns are never streamed, so no zero-memset and no wasted PE rows.
  - Denominator via ones-column in V^T (rides along in the AV matmul).
  - V projected directly in [t, d] orientation (x-chunk stationary), no PE
    transposes.
  - Deferred normalization: Y^T drained unnormalized per chunk; recip (DVE),
    partition-broadcast + multiply (Pool) in chunk-wide ops.
  - Out projection per chunk from normalized Y^T; PSUM staged to SBUF bf16
    (DVE/Act alternating) and DMA'd; host sums the 8 partial outputs.
  - Manual interleave: proj/out-proj matmuls woven between attention blocks so
    the PE never idles during the Act-bound exp phase.
"""

import sys

for p in ("/opt/trn_rl_repo", "/root/.axon_site/_ro/trn_rl_repo"):
    if p not in sys.path:
        sys.path.insert(0, p)

import numpy as np
import ml_dtypes
from collections import deque
from contextlib import ExitStack

import concourse.bacc as bacc
import concourse.mybir as mybir
import concourse.tile as tile

F32 = mybir.dt.float32
BF16 = mybir.dt.bfloat16
BF16_NP = ml_dtypes.bfloat16
EXP = mybir.ActivationFunctionType.Exp

D = 64
ROPE_BASE = 10000.0
AC = 512  # t-chunk


def build_nc(C, T, B):
    CT = C // 128          # contraction tiles (16)
    NCH = T // AC          # chunks per batch (4)
    BT = B * T
    KPB = T // 128         # key blocks per batch (16)

    nc = bacc.Bacc("TRN2", target_bir_lowering=False, debug=False)

    xTr = nc.dram_tensor("xTr", [128, CT, BT], BF16, kind="ExternalInput")
    wqr = nc.dram_tensor("wqr", [128, CT, 256], BF16, kind="ExternalInput")
    wkr = nc.dram_tensor("wkr", [128, CT, 64], BF16, kind="ExternalInput")
    wvr = nc.dram_tensor("wvr", [128, CT, 64], BF16, kind="ExternalInput")
    wor = nc.dram_tensor("wor", [128, 2, C], BF16, kind="ExternalInput")
    rqc = nc.dram_tensor("rqc", [128, T], BF16, kind="ExternalInput")
    rqs = nc.dram_tensor("rqs", [128, T], BF16, kind="ExternalInput")
    rkc = nc.dram_tensor("rkc", [64, T], BF16, kind="ExternalInput")
    rks = nc.dram_tensor("rks", [64, T], BF16, kind="ExternalInput")
    tri2 = nc.dram_tensor("tri2", [128, 2, 128], BF16, kind="ExternalInput")
    out = nc.dram_tensor("out", [BT, C], BF16, kind="ExternalOutput")

    with tile.TileContext(nc) as tc, ExitStack() as ctx:
        # PSUM: pj 2 + ss 4 + yy 2 = 8 banks
        pj = ctx.enter_context(tc.tile_pool(name="pj", bufs=2, space="PSUM"))
        ss = ctx.enter_context(tc.tile_pool(name="ss", bufs=2, space="PSUM"))
        yy = ctx.enter_context(tc.tile_pool(name="yy", bufs=1, space="PSUM"))

        cst = ctx.enter_context(tc.tile_pool(name="cst", bufs=1))
        xcp = ctx.enter_context(tc.tile_pool(name="xcp", bufs=3))
        ktp = ctx.enter_context(tc.tile_pool(name="ktp", bufs=2))
        vpp = ctx.enter_context(tc.tile_pool(name="vpp", bufs=2))
        qsp = ctx.enter_context(tc.tile_pool(name="qsp", bufs=4))
        qtp = ctx.enter_context(tc.tile_pool(name="qtp", bufs=8))
        tmp = ctx.enter_context(tc.tile_pool(name="tmp", bufs=4))
        ppp = ctx.enter_context(tc.tile_pool(name="ppp", bufs=4))
        ytu = ctx.enter_context(tc.tile_pool(name="ytu", bufs=2))
        ytp = ctx.enter_context(tc.tile_pool(name="ytp", bufs=4))
        osp = ctx.enter_context(tc.tile_pool(name="osp", bufs=4))
        ncp = ctx.enter_context(tc.tile_pool(name="ncp", bufs=4))

        # ---- constants ----
        XW = cst.tile([128, CT, 256], BF16, tag="XW")
        KW = cst.tile([128, CT, 64], BF16, tag="KW")
        VW = cst.tile([128, CT, 64], BF16, tag="VW")
        OW = cst.tile([128, 2, C], BF16, tag="OW")
        RQC = cst.tile([128, T], BF16, tag="RQC")
        RQS = cst.tile([128, T], BF16, tag="RQS")
        RKC = cst.tile([64, T], BF16, tag="RKC")
        RKS = cst.tile([64, T], BF16, tag="RKS")
        TRI = cst.tile([128, 2, 128], BF16, tag="TRI")

        PROJ_ROPES = {}
        XC = {}   # (b, ch) -> x chunk tile [128, CT, AC]
        KT = {}   # b -> [64, T]
        VP = {}   # b -> [128, KPB, 65]
        QT = {}   # (ch%2, h) -> [64, AC]
        YT = {}   # (b, cl) -> [128, T]

        copy_flip = [0]

        def stage_copy(dst, src):
            """PSUM->SBUF staging, alternating DVE / Act."""
            copy_flip[0] ^= 1
            if copy_flip[0]:
                nc.vector.tensor_copy(dst, src)
            else:
                nc.scalar.copy(dst, src)

        def emit_xdma(b, ch):
            t = xcp.tile([128, CT, AC], BF16, tag="XC", name=f"XC{b}_{ch}")
            nc.sync.dma_start(t[:, :, :], xTr[:, :, b * T + ch * AC:b * T + (ch + 1) * AC])
            XC[(b, ch)] = t

        def fillers_proj(b, ch):
            """Closures projecting chunk (b, ch): K, V, Q0, Q1."""
            xc = XC[(b, ch)]
            tcol = ch * AC
            res = []

            pk_box, pv_box = [], []
            ks_box, qs_box = [], {}

            def k_mm(c0):
                if c0 == 0:
                    pk_box.append(pj.tile([64, AC], F32, tag="pj", name=f"PK{b}_{ch}"))
                pk = pk_box[0]
                for c in range(c0, c0 + 4):
                    nc.tensor.matmul(pk[:], KW[:, c, :], xc[:, c, :],
                                     start=(c == 0), stop=(c == CT - 1))

            def k_copy():
                ks = qsp.tile([64, AC], BF16, tag="KS", name=f"KS{b}_{ch}")
                nc.vector.tensor_copy(ks[:], pk_box[0][:])
                ks_box.append(ks)

            def k_rope():
                ks = ks_box[0]
                kt = KT[b]
                t1 = tmp.tile([64, AC], BF16, tag="kt1")
                t2 = tmp.tile([64, AC], BF16, tag="kt2")
                nc.vector.tensor_mul(t1[:], ks[:], RKC[:, tcol:tcol + AC])
                nc.vector.tensor_mul(t2[0:32, :], ks[32:64, :], RKS[32:64, tcol:tcol + AC])
                nc.vector.tensor_mul(t2[32:64, :], ks[0:32, :], RKS[0:32, tcol:tcol + AC])
                nc.vector.tensor_add(kt[:, tcol:tcol + AC], t1[:], t2[:])

            def v_mm(tb):
                if tb == 0:
                    pv_box.append(pj.tile([128, 4, 64], F32, tag="pj", name=f"PV{b}_{ch}"))
                pv = pv_box[0]
                for c in range(CT):
                    nc.tensor.matmul(pv[:, tb, :],
                                     xc[:, c, tb * 128:(tb + 1) * 128], VW[:, c, :],
                                     start=(c == 0), stop=(c == CT - 1))

            def v_stage():
                nc.vector.tensor_copy(VP[b][:, ch * 4:(ch + 1) * 4, 0:64], pv_box[0][:, :, :])

            def q_mm(c0, hp, pq_box):
                if c0 == 0:
                    pq_box.append(pj.tile([128, AC], F32, tag="pj", name=f"PQ{b}_{ch}_{hp}"))
                pq = pq_box[0]
                for c in range(c0, c0 + 4):
                    nc.tensor.matmul(pq[:], XW[:, c, hp * 128:(hp + 1) * 128], xc[:, c, :],
                                     start=(c == 0), stop=(c == CT - 1))

            def q_copy(hp, pq_box):
                qs = qsp.tile([128, AC], BF16, tag="QS", name=f"QS{b}_{ch}_{hp}")
                nc.vector.tensor_copy(qs[:], pq_box[0][:])
                qs_box[hp] = qs

            def q_rope(hp):
                qs = qs_box[hp]
                t1 = tmp.tile([128, AC], BF16, tag="qt1")
                t2 = tmp.tile([128, AC], BF16, tag="qt2")
                nc.vector.tensor_mul(t1[:], qs[:], RQC[:, tcol:tcol + AC])
                for b0 in (0, 64):
                    nc.vector.tensor_mul(t2[b0:b0 + 32, :], qs[b0 + 32:b0 + 64, :],
                                         RQS[b0 + 32:b0 + 64, tcol:tcol + AC])
                    nc.vector.tensor_mul(t2[b0 + 32:b0 + 64, :], qs[b0:b0 + 32, :],
                                         RQS[b0:b0 + 32, tcol:tcol + AC])
                for hl in range(2):
                    h = 2 * hp + hl
                    qt = qtp.tile([64, AC], BF16, tag="QT", name=f"QT{b}_{ch}_{h}")
                    nc.vector.tensor_add(qt[:], t1[hl * 64:hl * 64 + 64, :],
                                         t2[hl * 64:hl * 64 + 64, :])
                    QT[(ch % 2, h)] = qt

            # psum->sbuf copies right behind each chain (frees pj bufs fast);
            # long DVE rope bursts deferred to the end
            for c0 in range(0, CT, 4):
                res.append((875, lambda c0=c0: k_mm(c0)))
            res.append((10, k_copy))
            for tb in range(4):
                res.append((430, lambda tb=tb: v_mm(tb)))
            res.append((10, v_stage))
            pq_boxes = [[], []]
            for hp in range(2):
                for c0 in range(0, CT, 4):
                    res.append((875, lambda c0=c0, hp=hp: q_mm(c0, hp, pq_boxes[hp])))
                res.append((10, lambda hp=hp: q_copy(hp, pq_boxes[hp])))
            ropes = [(150, k_rope), (150, lambda: q_rope(0)),
                     (150, lambda: q_rope(1))]
            return res, ropes

        TAIL = [False]

        def fillers_outproj(b, ch):
            """Closures for out projection of chunk (b, ch) (needs YT cols).

            Two co-columns per group share one [128,1024] staging tile and a
            single DMA — halves the serialized HWDGE holds (625ns each)."""
            res = []

            def po_group(tt, co2):
                trow = b * T + ch * AC + tt * 128
                os_ = osp.tile([128, 1024], BF16, tag="OS")
                for j in range(2):
                    co = 2 * co2 + j
                    po = pj.tile([128, 512], F32, tag="pj", name=f"PO{b}_{ch}_{tt}_{co}")
                    for cl in range(2):
                        nc.tensor.matmul(po[:], YT[(b, cl)][:, ch * 4 * 128 + tt * 128:ch * 4 * 128 + (tt + 1) * 128],
                                         OW[:, cl, co * 512:(co + 1) * 512],
                                         start=(cl == 0), stop=(cl == 1))
                    if j == 0:
                        nc.vector.tensor_copy(os_[:, 0:512], po[:])
                    else:
                        nc.scalar.copy(os_[:, 512:1024], po[:])
                nc.sync.dma_start(out[trow:trow + 128, co2 * 1024:(co2 + 1) * 1024], os_[:])

            for tt in range(4):
                for co2 in range(C // 1024):
                    res.append((880, lambda tt=tt, co2=co2: po_group(tt, co2)))
            return res

        projq = deque()   # (cost_ns, closure) — must drain before next chunk
        sideq = deque()   # out-proj groups — drain lazily
        qcost = [0]       # total cost queued

        def push(q, items):
            q.extend(items)
            qcost[0] += sum(c for c, _ in items)

        reserve = [0]

        def head_cost():
            if projq:
                return projq[0][0]
            if sideq and qcost[0] > reserve[0]:
                return sideq[0][0]
            return None

        def fill(budget):
            while budget > 0:
                if projq:
                    c, f = projq.popleft()
                elif sideq and qcost[0] > reserve[0]:
                    c, f = sideq.popleft()
                else:
                    return
                f()
                qcost[0] -= c
                budget -= c

        def flush_proj():
            while projq:
                c, f = projq.popleft()
                qcost[0] -= c
                f()

        def flush_all():
            flush_proj()
            while sideq:
                c, f = sideq.popleft()
                qcost[0] -= c
                f()

        def attention_chunk(b, ch, budget, last_chunk):
            """Both head-pair passes of chunk (b, ch) as one pipelined stream."""
            kt = KT[b]
            vp = VP[b]
            kis = [("d", l) for l in range(4)] + [("f", k) for k in range(ch * 4)]
            n = len(kis)
            ytuc = ytu.tile([65, 4, AC], BF16, tag="YTU", name=f"YTU{b}_{ch}")
            yps = {}
            pend = deque()

            def emit_av(e):
                p2, f0, pr, idx, ki_ = e
                for j in range(2):
                    nc.tensor.matmul(yps[pr][:, j, f0:512], vp[:, ki_, :], p2[:, j, f0:512],
                                     start=(idx == 0), stop=(idx == n - 1))
                if idx == n - 1:
                    if last_chunk and pr == 1:
                        normalize_direct(b, ch, yps[pr], pr)
                    else:
                        normalize(b, ch, ytuc, yps[pr], pr)

            for pr in range(2):
                for i, (kind, v) in enumerate(kis):
                    ki = ch * 4 + v if kind == "d" else v
                    f0 = v * 128 if kind == "d" else 0
                    if i == 0:
                        yps[pr] = yy.tile([65, 2, 512], F32, tag="yy", name=f"Y{b}_{ch}_{pr}")
                    s2 = ss.tile([128, 2, 512], F32, tag="ss", name=f"S{b}_{ch}_{pr}_{i}")
                    for j in range(2):
                        h = 2 * pr + j
                        nc.tensor.matmul(s2[:, j, f0:512], kt[:, ki * 128:(ki + 1) * 128],
                                         QT[(ch % 2, h)][:, f0:512], start=True, stop=True)
                    p2 = ppp.tile([128, 2, 512], BF16, tag="P2")
                    nc.scalar.activation(p2[:, :, f0:512], s2[:, :, f0:512], EXP)
                    if kind == "d":
                        nc.vector.tensor_mul(p2[:, :, f0:f0 + 128], p2[:, :, f0:f0 + 128], TRI[:, :, :])
                    pend.append((p2, f0, pr, i, ki))
                    if len(pend) > 2:
                        emit_av(pend.popleft())
                    fill(800)
            while pend:
                emit_av(pend.popleft())

        def normalize(b, ch, ytuc, yps, pr):
            """Normalize the two heads of pass pr into YT.

            Recips read the denominator rows straight from PSUM (ahead of the
            Y drain); the two multiplies run on different engines in parallel.
            """
            tcol = ch * AC
            nc.vector.tensor_copy(ytuc[:, 2 * pr:2 * pr + 2, :], yps[:, :, :])
            rcs = []
            for j in range(2):
                rc = ncp.tile([1, AC], F32, tag="RC")
                nc.vector.reciprocal(rc[0:1, :], ytuc[64:65, 2 * pr + j, :])
                rcs.append(rc)
            bcs = []
            for j in range(2):
                bc = ncp.tile([64, AC], F32, tag="BC")
                nc.gpsimd.partition_broadcast(bc[:], rcs[j][0:1, :])
                bcs.append(bc)
            for j in range(2):
                h = 2 * pr + j
                cl, r0 = h // 2, (h % 2) * 64
                nc.gpsimd.tensor_mul(YT[(b, cl)][r0:r0 + 64, tcol:tcol + AC],
                                     ytuc[0:64, h, :], bcs[j][:])

        def normalize_direct(b, ch, yps, pr):
            """Tail fast path: normalize straight from PSUM (no drain copy),
            DVE muls, h-pipelined recip/bcast."""
            tcol = ch * AC
            rcs, bcs = [], []
            for j in range(2):
                rc = ncp.tile([1, AC], F32, tag="RC")
                nc.vector.reciprocal(rc[0:1, :], yps[64:65, j, :])
                rcs.append(rc)
            for j in range(2):
                bc = ncp.tile([64, AC], F32, tag="BC")
                nc.gpsimd.partition_broadcast(bc[:], rcs[j][0:1, :])
                bcs.append(bc)
            for j in range(2):
                h = 2 * pr + j
                cl, r0 = h // 2, (h % 2) * 64
                nc.vector.tensor_mul(YT[(b, cl)][r0:r0 + 64, tcol:tcol + AC],
                                     yps[0:64, j, :], bcs[j][:])

        # ---- PE warmup: ramp the p-state while the first DMAs land ----
        WRM = cst.tile([128, 512], BF16, tag="WRM")
        nc.vector.memset(WRM[:], 0.0)
        pwarm = pj.tile([128, 512], F32, tag="pj", name="PWARM")
        for _ in range(18):
            nc.tensor.matmul(pwarm[:], WRM[:, 0:128], WRM[:], start=True, stop=True)

        # ---- preamble DMAs (need-ordered; x chunk 0 split so K proj can
        #      start as soon as its weights + first c-rows arrive) ----
        x00 = xcp.tile([128, CT, AC], BF16, tag="XC", name="XC0_0")
        XC[(0, 0)] = x00
        nc.sync.dma_start(KW[:, 0:8, :], wkr[:, 0:8, :])
        nc.sync.dma_start(x00[:, 0:4, :], xTr[:, 0:4, 0:AC])
        nc.sync.dma_start(KW[:, 8:CT, :], wkr[:, 8:CT, :])
        for q in range(1, 4):
            nc.sync.dma_start(x00[:, 4 * q:4 * (q + 1), :],
                              xTr[:, 4 * q:4 * (q + 1), 0:AC])
        nc.sync.dma_start(RKC[:], rkc[:])
        nc.sync.dma_start(RKS[:], rks[:])
        nc.sync.dma_start(VW[:, :, :], wvr[:, :, :])
        nc.sync.dma_start(XW[:, :, :], wqr[:, :, :])
        nc.sync.dma_start(RQC[:], rqc[:])
        nc.sync.dma_start(RQS[:], rqs[:])
        nc.sync.dma_start(TRI[:, :, :], tri2[:, :, :])
        nc.sync.dma_start(OW[:, :, :], wor[:, :, :])
        emit_xdma(0, 1)

        for b in range(B):
            KT[b] = ktp.tile([64, T], BF16, tag="KT", name=f"KT{b}")
            VP[b] = vpp.tile([128, KPB, 65], BF16, tag="VP", name=f"VP{b}")
            nc.vector.memset(VP[b][:, :, 64:65], 1.0)
            for cl in range(2):
                YT[(b, cl)] = ytp.tile([128, T], BF16, tag="YT", name=f"YT{b}_{cl}")

        ca, ra = fillers_proj(0, 0)
        for _, c in ca + ra:
            c()

        def succ(b, ch, k):
            t = b * NCH + ch + k
            return (t // NCH, t % NCH) if t < B * NCH else None

        pushed = set()
        for b in range(B):
            for ch in range(NCH):
                flush_proj()  # this chunk's proj must be complete
                nxt = succ(b, ch, 2)
                if nxt:
                    emit_xdma(*nxt)
                # queue fillers: chains for chunk+1 (if new) + its ropes,
                # then chains of chunk+2 (QT-parity-safe extra supply)
                nxt = succ(b, ch, 1)
                if nxt:
                    if nxt not in pushed:
                        ca, ra = fillers_proj(*nxt)
                        push(projq, ca)
                        pushed.add(nxt)
                        PROJ_ROPES[nxt] = ra
                    push(projq, PROJ_ROPES.pop(nxt))
                nxt = succ(b, ch, 2)
                if ch >= 2 and nxt and nxt not in pushed:
                    ca, ra = fillers_proj(*nxt)
                    push(projq, ca)
                    pushed.add(nxt)
                    PROJ_ROPES[nxt] = ra
                last = b == B - 1 and ch == NCH - 1
                reserve[0] = 4000 if last else 0
                attention_chunk(b, ch, 800, last)
                push(sideq, fillers_outproj(b, ch))
        TAIL[0] = True
        flush_all()

    nc.compile()
    return nc


def rope_tables(T, scale):
    inv = 1.0 / (ROPE_BASE ** (np.arange(0, D, 2, dtype=np.float32) / D))
    t = np.arange(T, dtype=np.float32)
    freqs = np.outer(t, inv)
    emb = np.concatenate([freqs, freqs], -1)
    cos = np.cos(emb).T.astype(np.float32) * scale
    sin = np.sin(emb).T.astype(np.float32) * scale
    sinX = np.empty((64, T), np.float32)
    sinX[0:32] = sin[32:64]
    sinX[32:64] = -sin[0:32]
    return np.ascontiguousarray(cos), np.ascontiguousarray(sinX)


def _pk(a, nblk):
    """[nblk*128, F] -> [128, nblk, F] contiguous bf16."""
    n, f = a.shape
    return np.ascontiguousarray(
        a.reshape(nblk, 128, f).transpose(1, 0, 2)).astype(BF16_NP)


def make_inputs(x, Wq, Wk, Wv, Wo):
    B, T, C = x.shape
    CT = C // 128
    xT = np.ascontiguousarray(x.reshape(B * T, C).T)
    qc, qs = rope_tables(T, 1.0 / np.sqrt(D).astype(np.float32))
    kc, ks = rope_tables(T, 1.0)
    tri = np.triu(np.ones((128, 128), np.float32))
    common = {
        "xTr": _pk(xT, CT),
        "rqc": np.concatenate([qc, qc], 0).astype(BF16_NP),
        "rqs": np.concatenate([qs, qs], 0).astype(BF16_NP),
        "rkc": kc.astype(BF16_NP),
        "rks": ks.astype(BF16_NP),
        "tri2": np.ascontiguousarray(
            np.stack([tri, tri], 1)).astype(BF16_NP),
    }
    in_maps = []
    for i in range(8):
        m = dict(common)
        m["wqr"] = _pk(np.ascontiguousarray(Wq[:, i * 256:(i + 1) * 256]), CT)
        m["wkr"] = _pk(np.ascontiguousarray(Wk[:, i * 64:(i + 1) * 64]), CT)
        m["wvr"] = _pk(np.ascontiguousarray(Wv[:, i * 64:(i + 1) * 64]), CT)
        m["wor"] = _pk(np.ascontiguousarray(Wo[i * 256:(i + 1) * 256, :]), 2)
        in_maps.append(m)
    return in_maps


_NC_CACHE = {}


def _get_nc(C, T, B):
    key = (C, T, B)
    if key not in _NC_CACHE:
        _NC_CACHE[key] = build_nc(C, T, B)
    return _NC_CACHE[key]


def run(x, Wq, Wk, Wv, Wo, trace=False):
    from concourse.bass_utils import run_bass_kernel_spmd

    B, T, C = x.shape
    nc = _get_nc(C, T, B)
    in_maps = make_inputs(x, Wq, Wk, Wv, Wo)
    for attempt in range(3):
        try:
            res = run_bass_kernel_spmd(nc, in_maps, list(range(8)), trace=trace)
        except (ImportError, ModuleNotFoundError):
            res = run_bass_kernel_spmd(nc, in_maps, list(range(8)), trace=False)
        acc = res.results[0]["out"].astype(np.float32)
        for i in range(1, 8):
            acc = acc + res.results[i]["out"].astype(np.float32)
        if np.isfinite(acc).all():
            break
    return acc.reshape(B, T, C), res


def kernel(x, Wq, Wk, Wv, Wo):
    out, _ = run(x, Wq, Wk, Wv, Wo, trace=False)
    return out


# revision 71
# speedup vs baseline: 1.0284x; 1.0002x over previous
"""GQA kernel for TRN2, 8-way tensor-parallel by KV head (v2).

Per core i: KV head i, Q heads 4i..4i+3. All matmuls bf16 (full PE rate at any
free size). Cost-model-driven design:
  - Coalesced DMAs: host pre-lays x^T as [128, 16, B*T] so each 512-col chunk
    loads in ONE descriptor-dense DMA (HWDGE hold is ~625ns per DMA).
  - Scores S^T = K Q^T per 128-key block, two heads side-by-side in one
    [128, 2, 512] PSUM duo tile; ONE exp per block over both heads via 3-D AP,
    diagonal blocks trimmed to the causally valid column window.
  - Causality: diagonal-first AV accumulation with subrange matmuls — invalid
    columns are never streamed, so no zero-memset and no wasted PE rows.
  - Denominator via ones-column in V^T (rides along in the AV matmul).
  - V projected directly in [t, d] orientation (x-chunk stationary), no PE
    transposes.
  - Deferred normalization: Y^T drained unnormalized per chunk; recip (DVE),
    partition-broadcast + multiply (Pool) in chunk-wide ops.
  - Out projection per chunk from normalized Y^T; PSUM staged to SBUF bf16
    (DVE/Act alternating) and DMA'd; host sums the 8 partial outputs.
  - Manual interleave: proj/out-proj matmuls woven between attention blocks so
    the PE never idles during the Act-bound exp phase.
"""

import sys

for p in ("/opt/trn_rl_repo", "/root/.axon_site/_ro/trn_rl_repo"):
    if p not in sys.path:
        sys.path.insert(0, p)

import numpy as np
import ml_dtypes
from collections import deque
from contextlib import ExitStack

import concourse.bacc as bacc
import concourse.mybir as mybir
import concourse.tile as tile

F32 = mybir.dt.float32
BF16 = mybir.dt.bfloat16
BF16_NP = ml_dtypes.bfloat16
EXP = mybir.ActivationFunctionType.Exp

D = 64
ROPE_BASE = 10000.0
AC = 512  # t-chunk


def build_nc(C, T, B):
    CT = C // 128          # contraction tiles (16)
    NCH = T // AC          # chunks per batch (4)
    BT = B * T
    KPB = T // 128         # key blocks per batch (16)

    nc = bacc.Bacc("TRN2", target_bir_lowering=False, debug=False)

    xTr = nc.dram_tensor("xTr", [128, CT, BT], BF16, kind="ExternalInput")
    wqr = nc.dram_tensor("wqr", [128, CT, 256], BF16, kind="ExternalInput")
    wkr = nc.dram_tensor("wkr", [128, CT, 64], BF16, kind="ExternalInput")
    wvr = nc.dram_tensor("wvr", [128, CT, 64], BF16, kind="ExternalInput")
    wor = nc.dram_tensor("wor", [128, 2, C], BF16, kind="ExternalInput")
    rqc = nc.dram_tensor("rqc", [128, T], BF16, kind="ExternalInput")
    rqs = nc.dram_tensor("rqs", [128, T], BF16, kind="ExternalInput")
    rkc = nc.dram_tensor("rkc", [64, T], BF16, kind="ExternalInput")
    rks = nc.dram_tensor("rks", [64, T], BF16, kind="ExternalInput")
    tri2 = nc.dram_tensor("tri2", [128, 2, 128], BF16, kind="ExternalInput")
    out = nc.dram_tensor("out", [BT, C], BF16, kind="ExternalOutput")

    with tile.TileContext(nc) as tc, ExitStack() as ctx:
        # PSUM: pj 2 + ss 4 + yy 2 = 8 banks
        pj = ctx.enter_context(tc.tile_pool(name="pj", bufs=2, space="PSUM"))
        ss = ctx.enter_context(tc.tile_pool(name="ss", bufs=2, space="PSUM"))
        yy = ctx.enter_context(tc.tile_pool(name="yy", bufs=1, space="PSUM"))

        cst = ctx.enter_context(tc.tile_pool(name="cst", bufs=1))
        xcp = ctx.enter_context(tc.tile_pool(name="xcp", bufs=3))
        ktp = ctx.enter_context(tc.tile_pool(name="ktp", bufs=2))
        vpp = ctx.enter_context(tc.tile_pool(name="vpp", bufs=2))
        qsp = ctx.enter_context(tc.tile_pool(name="qsp", bufs=4))
        qtp = ctx.enter_context(tc.tile_pool(name="qtp", bufs=8))
        tmp = ctx.enter_context(tc.tile_pool(name="tmp", bufs=4))
        ppp = ctx.enter_context(tc.tile_pool(name="ppp", bufs=4))
        ytu = ctx.enter_context(tc.tile_pool(name="ytu", bufs=2))
        ytp = ctx.enter_context(tc.tile_pool(name="ytp", bufs=4))
        osp = ctx.enter_context(tc.tile_pool(name="osp", bufs=4))
        ncp = ctx.enter_context(tc.tile_pool(name="ncp", bufs=4))

        # ---- constants ----
        XW = cst.tile([128, CT, 256], BF16, tag="XW")
        KW = cst.tile([128, CT, 64], BF16, tag="KW")
        VW = cst.tile([128, CT, 64], BF16, tag="VW")
        OW = cst.tile([128, 2, C], BF16, tag="OW")
        RQC = cst.tile([128, T], BF16, tag="RQC")
        RQS = cst.tile([128, T], BF16, tag="RQS")
        RKC = cst.tile([64, T], BF16, tag="RKC")
        RKS = cst.tile([64, T], BF16, tag="RKS")
        TRI = cst.tile([128, 2, 128], BF16, tag="TRI")

        PROJ_ROPES = {}
        XC = {}   # (b, ch) -> x chunk tile [128, CT, AC]
        KT = {}   # b -> [64, T]
        VP = {}   # b -> [128, KPB, 65]
        QT = {}   # (ch%2, h) -> [64, AC]
        YT = {}   # (b, cl) -> [128, T]

        copy_flip = [0]

        def stage_copy(dst, src):
            """PSUM->SBUF staging, alternating DVE / Act."""
            copy_flip[0] ^= 1
            if copy_flip[0]:
                nc.vector.tensor_copy(dst, src)
            else:
                nc.scalar.copy(dst, src)

        def emit_xdma(b, ch):
            t = xcp.tile([128, CT, AC], BF16, tag="XC", name=f"XC{b}_{ch}")
            nc.sync.dma_start(t[:, :, :], xTr[:, :, b * T + ch * AC:b * T + (ch + 1) * AC])
            XC[(b, ch)] = t

        def fillers_proj(b, ch):
            """Closures projecting chunk (b, ch): K, V, Q0, Q1."""
            xc = XC[(b, ch)]
            tcol = ch * AC
            res = []

            pk_box, pv_box = [], []
            ks_box, qs_box = [], {}

            def k_mm(c0):
                if c0 == 0:
                    pk_box.append(pj.tile([64, AC], F32, tag="pj", name=f"PK{b}_{ch}"))
                pk = pk_box[0]
                for c in range(c0, c0 + 4):
                    nc.tensor.matmul(pk[:], KW[:, c, :], xc[:, c, :],
                                     start=(c == 0), stop=(c == CT - 1))

            def k_copy():
                ks = qsp.tile([64, AC], BF16, tag="KS", name=f"KS{b}_{ch}")
                nc.vector.tensor_copy(ks[:], pk_box[0][:])
                ks_box.append(ks)

            def k_rope():
                ks = ks_box[0]
                kt = KT[b]
                t1 = tmp.tile([64, AC], BF16, tag="kt1")
                t2 = tmp.tile([64, AC], BF16, tag="kt2")
                nc.vector.tensor_mul(t1[:], ks[:], RKC[:, tcol:tcol + AC])
                nc.vector.tensor_mul(t2[0:32, :], ks[32:64, :], RKS[32:64, tcol:tcol + AC])
                nc.vector.tensor_mul(t2[32:64, :], ks[0:32, :], RKS[0:32, tcol:tcol + AC])
                nc.vector.tensor_add(kt[:, tcol:tcol + AC], t1[:], t2[:])

            def v_mm(tb):
                if tb == 0:
                    pv_box.append(pj.tile([128, 4, 64], F32, tag="pj", name=f"PV{b}_{ch}"))
                pv = pv_box[0]
                for c in range(CT):
                    nc.tensor.matmul(pv[:, tb, :],
                                     xc[:, c, tb * 128:(tb + 1) * 128], VW[:, c, :],
                                     start=(c == 0), stop=(c == CT - 1))

            def v_stage():
                nc.vector.tensor_copy(VP[b][:, ch * 4:(ch + 1) * 4, 0:64], pv_box[0][:, :, :])

            def q_mm(c0, hp, pq_box):
                if c0 == 0:
                    pq_box.append(pj.tile([128, AC], F32, tag="pj", name=f"PQ{b}_{ch}_{hp}"))
                pq = pq_box[0]
                for c in range(c0, c0 + 4):
                    nc.tensor.matmul(pq[:], XW[:, c, hp * 128:(hp + 1) * 128], xc[:, c, :],
                                     start=(c == 0), stop=(c == CT - 1))

            def q_copy(hp, pq_box):
                qs = qsp.tile([128, AC], BF16, tag="QS", name=f"QS{b}_{ch}_{hp}")
                nc.vector.tensor_copy(qs[:], pq_box[0][:])
                qs_box[hp] = qs

            def q_rope(hp):
                qs = qs_box[hp]
                t1 = tmp.tile([128, AC], BF16, tag="qt1")
                t2 = tmp.tile([128, AC], BF16, tag="qt2")
                nc.vector.tensor_mul(t1[:], qs[:], RQC[:, tcol:tcol + AC])
                for b0 in (0, 64):
                    nc.vector.tensor_mul(t2[b0:b0 + 32, :], qs[b0 + 32:b0 + 64, :],
                                         RQS[b0 + 32:b0 + 64, tcol:tcol + AC])
                    nc.vector.tensor_mul(t2[b0 + 32:b0 + 64, :], qs[b0:b0 + 32, :],
                                         RQS[b0:b0 + 32, tcol:tcol + AC])
                for hl in range(2):
                    h = 2 * hp + hl
                    qt = qtp.tile([64, AC], BF16, tag="QT", name=f"QT{b}_{ch}_{h}")
                    nc.vector.tensor_add(qt[:], t1[hl * 64:hl * 64 + 64, :],
                                         t2[hl * 64:hl * 64 + 64, :])
                    QT[(ch % 2, h)] = qt

            # psum->sbuf copies right behind each chain (frees pj bufs fast);
            # long DVE rope bursts deferred to the end
            for c0 in range(0, CT, 4):
                res.append((875, lambda c0=c0: k_mm(c0)))
            res.append((10, k_copy))
            for tb in range(4):
                res.append((430, lambda tb=tb: v_mm(tb)))
            res.append((10, v_stage))
            pq_boxes = [[], []]
            for hp in range(2):
                for c0 in range(0, CT, 4):
                    res.append((875, lambda c0=c0, hp=hp: q_mm(c0, hp, pq_boxes[hp])))
                res.append((10, lambda hp=hp: q_copy(hp, pq_boxes[hp])))
            ropes = [(150, k_rope), (150, lambda: q_rope(0)),
                     (150, lambda: q_rope(1))]
            return res, ropes

        TAIL = [False]

        def fillers_outproj(b, ch):
            """Closures for out projection of chunk (b, ch) (needs YT cols).

            Two co-columns per group share one [128,1024] staging tile and a
            single DMA — halves the serialized HWDGE holds (625ns each)."""
            res = []

            os_box = {}

            def po_group(tt, co2):
                trow = b * T + ch * AC + tt * 128
                if co2 == 0:
                    os_box[tt] = osp.tile([128, 2048], BF16, tag="OS", name=f"OS{b}_{ch}_{tt}")
                os_ = os_box[tt]
                for j in range(2):
                    co = 2 * co2 + j
                    po = pj.tile([128, 512], F32, tag="pj", name=f"PO{b}_{ch}_{tt}_{co}")
                    for cl in range(2):
                        nc.tensor.matmul(po[:], YT[(b, cl)][:, ch * 4 * 128 + tt * 128:ch * 4 * 128 + (tt + 1) * 128],
                                         OW[:, cl, co * 512:(co + 1) * 512],
                                         start=(cl == 0), stop=(cl == 1))
                    if j == 0:
                        nc.vector.tensor_copy(os_[:, co2 * 1024:co2 * 1024 + 512], po[:])
                    else:
                        nc.scalar.copy(os_[:, co2 * 1024 + 512:co2 * 1024 + 1024], po[:])
                if co2 == 1:
                    nc.sync.dma_start(out[trow:trow + 128, :], os_[:])

            for tt in range(4):
                for co2 in range(C // 1024):
                    res.append((880, lambda tt=tt, co2=co2: po_group(tt, co2)))
            return res

        projq = deque()   # (cost_ns, closure) — must drain before next chunk
        sideq = deque()   # out-proj groups — drain lazily
        qcost = [0]       # total cost queued

        def push(q, items):
            q.extend(items)
            qcost[0] += sum(c for c, _ in items)

        reserve = [0]

        def head_cost():
            if projq:
                return projq[0][0]
            if sideq and qcost[0] > reserve[0]:
                return sideq[0][0]
            return None

        def fill(budget):
            while budget > 0:
                if projq:
                    c, f = projq.popleft()
                elif sideq and qcost[0] > reserve[0]:
                    c, f = sideq.popleft()
                else:
                    return
                f()
                qcost[0] -= c
                budget -= c

        def flush_proj():
            while projq:
                c, f = projq.popleft()
                qcost[0] -= c
                f()

        def flush_all():
            flush_proj()
            while sideq:
                c, f = sideq.popleft()
                qcost[0] -= c
                f()

        def attention_chunk(b, ch, budget, last_chunk):
            """Both head-pair passes of chunk (b, ch) as one pipelined stream."""
            kt = KT[b]
            vp = VP[b]
            kis = [("d", l) for l in range(4)] + [("f", k) for k in range(ch * 4)]
            n = len(kis)
            ytuc = ytu.tile([65, 4, AC], BF16, tag="YTU", name=f"YTU{b}_{ch}")
            yps = {}
            pend = deque()

            def emit_av(e):
                p2, f0, pr, idx, ki_ = e
                for j in range(2):
                    nc.tensor.matmul(yps[pr][:, j, f0:512], vp[:, ki_, :], p2[:, j, f0:512],
                                     start=(idx == 0), stop=(idx == n - 1))
                if idx == n - 1:
                    if last_chunk and pr == 1:
                        normalize_direct(b, ch, yps[pr], pr)
                    else:
                        normalize(b, ch, ytuc, yps[pr], pr)

            for pr in range(2):
                for i, (kind, v) in enumerate(kis):
                    ki = ch * 4 + v if kind == "d" else v
                    f0 = v * 128 if kind == "d" else 0
                    if i == 0:
                        yps[pr] = yy.tile([65, 2, 512], F32, tag="yy", name=f"Y{b}_{ch}_{pr}")
                    s2 = ss.tile([128, 2, 512], F32, tag="ss", name=f"S{b}_{ch}_{pr}_{i}")
                    for j in range(2):
                        h = 2 * pr + j
                        nc.tensor.matmul(s2[:, j, f0:512], kt[:, ki * 128:(ki + 1) * 128],
                                         QT[(ch % 2, h)][:, f0:512], start=True, stop=True)
                    p2 = ppp.tile([128, 2, 512], BF16, tag="P2")
                    nc.scalar.activation(p2[:, :, f0:512], s2[:, :, f0:512], EXP)
                    if kind == "d":
                        nc.vector.tensor_mul(p2[:, :, f0:f0 + 128], p2[:, :, f0:f0 + 128], TRI[:, :, :])
                    pend.append((p2, f0, pr, i, ki))
                    if len(pend) > 2:
                        emit_av(pend.popleft())
                    fill(800)
            while pend:
                emit_av(pend.popleft())

        def normalize(b, ch, ytuc, yps, pr):
            """Normalize the two heads of pass pr into YT.

            Recips read the denominator rows straight from PSUM (ahead of the
            Y drain); the two multiplies run on different engines in parallel.
            """
            tcol = ch * AC
            nc.vector.tensor_copy(ytuc[:, 2 * pr:2 * pr + 2, :], yps[:, :, :])
            rcs = []
            for j in range(2):
                rc = ncp.tile([1, AC], F32, tag="RC")
                nc.vector.reciprocal(rc[0:1, :], ytuc[64:65, 2 * pr + j, :])
                rcs.append(rc)
            bcs = []
            for j in range(2):
                bc = ncp.tile([64, AC], F32, tag="BC")
                nc.gpsimd.partition_broadcast(bc[:], rcs[j][0:1, :])
                bcs.append(bc)
            for j in range(2):
                h = 2 * pr + j
                cl, r0 = h // 2, (h % 2) * 64
                nc.gpsimd.tensor_mul(YT[(b, cl)][r0:r0 + 64, tcol:tcol + AC],
                                     ytuc[0:64, h, :], bcs[j][:])

        def normalize_direct(b, ch, yps, pr):
            """Tail fast path: normalize straight from PSUM (no drain copy),
            DVE muls, h-pipelined recip/bcast."""
            tcol = ch * AC
            rcs, bcs = [], []
            for j in range(2):
                rc = ncp.tile([1, AC], F32, tag="RC")
                nc.vector.reciprocal(rc[0:1, :], yps[64:65, j, :])
                rcs.append(rc)
            for j in range(2):
                bc = ncp.tile([64, AC], F32, tag="BC")
                nc.gpsimd.partition_broadcast(bc[:], rcs[j][0:1, :])
                bcs.append(bc)
            for j in range(2):
                h = 2 * pr + j
                cl, r0 = h // 2, (h % 2) * 64
                nc.vector.tensor_mul(YT[(b, cl)][r0:r0 + 64, tcol:tcol + AC],
                                     yps[0:64, j, :], bcs[j][:])

        # ---- PE warmup: ramp the p-state while the first DMAs land ----
        WRM = cst.tile([128, 512], BF16, tag="WRM")
        nc.vector.memset(WRM[:], 0.0)
        pwarm = pj.tile([128, 512], F32, tag="pj", name="PWARM")
        for _ in range(18):
            nc.tensor.matmul(pwarm[:], WRM[:, 0:128], WRM[:], start=True, stop=True)

        # ---- preamble DMAs (need-ordered; x chunk 0 split so K proj can
        #      start as soon as its weights + first c-rows arrive) ----
        x00 = xcp.tile([128, CT, AC], BF16, tag="XC", name="XC0_0")
        XC[(0, 0)] = x00
        nc.sync.dma_start(KW[:, 0:8, :], wkr[:, 0:8, :])
        nc.sync.dma_start(x00[:, 0:4, :], xTr[:, 0:4, 0:AC])
        nc.sync.dma_start(KW[:, 8:CT, :], wkr[:, 8:CT, :])
        for q in range(1, 4):
            nc.sync.dma_start(x00[:, 4 * q:4 * (q + 1), :],
                              xTr[:, 4 * q:4 * (q + 1), 0:AC])
        nc.sync.dma_start(RKC[:], rkc[:])
        nc.sync.dma_start(RKS[:], rks[:])
        nc.sync.dma_start(VW[:, :, :], wvr[:, :, :])
        nc.sync.dma_start(XW[:, :, :], wqr[:, :, :])
        nc.sync.dma_start(RQC[:], rqc[:])
        nc.sync.dma_start(RQS[:], rqs[:])
        nc.sync.dma_start(TRI[:, :, :], tri2[:, :, :])
        nc.sync.dma_start(OW[:, :, :], wor[:, :, :])
        emit_xdma(0, 1)

        for b in range(B):
            KT[b] = ktp.tile([64, T], BF16, tag="KT", name=f"KT{b}")
            VP[b] = vpp.tile([128, KPB, 65], BF16, tag="VP", name=f"VP{b}")
            nc.vector.memset(VP[b][:, :, 64:65], 1.0)
            for cl in range(2):
                YT[(b, cl)] = ytp.tile([128, T], BF16, tag="YT", name=f"YT{b}_{cl}")

        ca, ra = fillers_proj(0, 0)
        for _, c in ca + ra:
            c()

        def succ(b, ch, k):
            t = b * NCH + ch + k
            return (t // NCH, t % NCH) if t < B * NCH else None

        pushed = set()
        for b in range(B):
            for ch in range(NCH):
                flush_proj()  # this chunk's proj must be complete
                nxt = succ(b, ch, 2)
                if nxt:
                    emit_xdma(*nxt)
                # queue fillers: chains for chunk+1 (if new) + its ropes,
                # then chains of chunk+2 (QT-parity-safe extra supply)
                nxt = succ(b, ch, 1)
                if nxt:
                    if nxt not in pushed:
                        ca, ra = fillers_proj(*nxt)
                        push(projq, ca)
                        pushed.add(nxt)
                        PROJ_ROPES[nxt] = ra
                    push(projq, PROJ_ROPES.pop(nxt))
                nxt = succ(b, ch, 2)
                if ch >= 2 and nxt and nxt not in pushed:
                    ca, ra = fillers_proj(*nxt)
                    push(projq, ca)
                    pushed.add(nxt)
                    PROJ_ROPES[nxt] = ra
                last = b == B - 1 and ch == NCH - 1
                reserve[0] = 4000 if last else 0
                attention_chunk(b, ch, 800, last)
                push(sideq, fillers_outproj(b, ch))
        TAIL[0] = True
        flush_all()

    nc.compile()
    return nc


def rope_tables(T, scale):
    inv = 1.0 / (ROPE_BASE ** (np.arange(0, D, 2, dtype=np.float32) / D))
    t = np.arange(T, dtype=np.float32)
    freqs = np.outer(t, inv)
    emb = np.concatenate([freqs, freqs], -1)
    cos = np.cos(emb).T.astype(np.float32) * scale
    sin = np.sin(emb).T.astype(np.float32) * scale
    sinX = np.empty((64, T), np.float32)
    sinX[0:32] = sin[32:64]
    sinX[32:64] = -sin[0:32]
    return np.ascontiguousarray(cos), np.ascontiguousarray(sinX)


def _pk(a, nblk):
    """[nblk*128, F] -> [128, nblk, F] contiguous bf16."""
    n, f = a.shape
    return np.ascontiguousarray(
        a.reshape(nblk, 128, f).transpose(1, 0, 2)).astype(BF16_NP)


def make_inputs(x, Wq, Wk, Wv, Wo):
    B, T, C = x.shape
    CT = C // 128
    xT = np.ascontiguousarray(x.reshape(B * T, C).T)
    qc, qs = rope_tables(T, 1.0 / np.sqrt(D).astype(np.float32))
    kc, ks = rope_tables(T, 1.0)
    tri = np.triu(np.ones((128, 128), np.float32))
    common = {
        "xTr": _pk(xT, CT),
        "rqc": np.concatenate([qc, qc], 0).astype(BF16_NP),
        "rqs": np.concatenate([qs, qs], 0).astype(BF16_NP),
        "rkc": kc.astype(BF16_NP),
        "rks": ks.astype(BF16_NP),
        "tri2": np.ascontiguousarray(
            np.stack([tri, tri], 1)).astype(BF16_NP),
    }
    in_maps = []
    for i in range(8):
        m = dict(common)
        m["wqr"] = _pk(np.ascontiguousarray(Wq[:, i * 256:(i + 1) * 256]), CT)
        m["wkr"] = _pk(np.ascontiguousarray(Wk[:, i * 64:(i + 1) * 64]), CT)
        m["wvr"] = _pk(np.ascontiguousarray(Wv[:, i * 64:(i + 1) * 64]), CT)
        m["wor"] = _pk(np.ascontiguousarray(Wo[i * 256:(i + 1) * 256, :]), 2)
        in_maps.append(m)
    return in_maps


_NC_CACHE = {}


def _get_nc(C, T, B):
    key = (C, T, B)
    if key not in _NC_CACHE:
        _NC_CACHE[key] = build_nc(C, T, B)
    return _NC_CACHE[key]


def run(x, Wq, Wk, Wv, Wo, trace=False):
    from concourse.bass_utils import run_bass_kernel_spmd

    B, T, C = x.shape
    nc = _get_nc(C, T, B)
    in_maps = make_inputs(x, Wq, Wk, Wv, Wo)
    for attempt in range(3):
        try:
            res = run_bass_kernel_spmd(nc, in_maps, list(range(8)), trace=trace)
        except (ImportError, ModuleNotFoundError):
            res = run_bass_kernel_spmd(nc, in_maps, list(range(8)), trace=False)
        acc = res.results[0]["out"].astype(np.float32)
        for i in range(1, 8):
            acc = acc + res.results[i]["out"].astype(np.float32)
        if np.isfinite(acc).all():
            break
    return acc.reshape(B, T, C), res


def kernel(x, Wq, Wk, Wv, Wo):
    out, _ = run(x, Wq, Wk, Wv, Wo, trace=False)
    return out


# revision 72
# speedup vs baseline: 1.0306x; 1.0021x over previous
"""GQA kernel for TRN2, 8-way tensor-parallel by KV head (v2).

Per core i: KV head i, Q heads 4i..4i+3. All matmuls bf16 (full PE rate at any
free size). Cost-model-driven design:
  - Coalesced DMAs: host pre-lays x^T as [128, 16, B*T] so each 512-col chunk
    loads in ONE descriptor-dense DMA (HWDGE hold is ~625ns per DMA).
  - Scores S^T = K Q^T per 128-key block, two heads side-by-side in one
    [128, 2, 512] PSUM duo tile; ONE exp per block over both heads via 3-D AP,
    diagonal blocks trimmed to the causally valid column window.
  - Causality: diagonal-first AV accumulation with subrange matmuls — invalid
    columns are never streamed, so no zero-memset and no wasted PE rows.
  - Denominator via ones-column in V^T (rides along in the AV matmul).
  - V projected directly in [t, d] orientation (x-chunk stationary), no PE
    transposes.
  - Deferred normalization: Y^T drained unnormalized per chunk; recip (DVE),
    partition-broadcast + multiply (Pool) in chunk-wide ops.
  - Out projection per chunk from normalized Y^T; PSUM staged to SBUF bf16
    (DVE/Act alternating) and DMA'd; host sums the 8 partial outputs.
  - Manual interleave: proj/out-proj matmuls woven between attention blocks so
    the PE never idles during the Act-bound exp phase.
"""

import sys

for p in ("/opt/trn_rl_repo", "/root/.axon_site/_ro/trn_rl_repo"):
    if p not in sys.path:
        sys.path.insert(0, p)

import numpy as np
import ml_dtypes
from collections import deque
from contextlib import ExitStack

import concourse.bacc as bacc
import concourse.mybir as mybir
import concourse.tile as tile

F32 = mybir.dt.float32
BF16 = mybir.dt.bfloat16
BF16_NP = ml_dtypes.bfloat16
EXP = mybir.ActivationFunctionType.Exp

D = 64
ROPE_BASE = 10000.0
AC = 512  # t-chunk


def build_nc(C, T, B):
    CT = C // 128          # contraction tiles (16)
    NCH = T // AC          # chunks per batch (4)
    BT = B * T
    KPB = T // 128         # key blocks per batch (16)

    nc = bacc.Bacc("TRN2", target_bir_lowering=False, debug=False)

    xTr = nc.dram_tensor("xTr", [128, CT, BT], BF16, kind="ExternalInput")
    wqr = nc.dram_tensor("wqr", [128, CT, 256], BF16, kind="ExternalInput")
    wkr = nc.dram_tensor("wkr", [128, CT, 64], BF16, kind="ExternalInput")
    wvr = nc.dram_tensor("wvr", [128, CT, 64], BF16, kind="ExternalInput")
    wor = nc.dram_tensor("wor", [128, 2, C], BF16, kind="ExternalInput")
    rqc = nc.dram_tensor("rqc", [128, T], BF16, kind="ExternalInput")
    rqs = nc.dram_tensor("rqs", [128, T], BF16, kind="ExternalInput")
    rkc = nc.dram_tensor("rkc", [64, T], BF16, kind="ExternalInput")
    rks = nc.dram_tensor("rks", [64, T], BF16, kind="ExternalInput")
    tri2 = nc.dram_tensor("tri2", [128, 2, 128], BF16, kind="ExternalInput")
    out = nc.dram_tensor("out", [BT, C], BF16, kind="ExternalOutput")

    with tile.TileContext(nc) as tc, ExitStack() as ctx:
        # PSUM: pj 2 + ss 4 + yy 2 = 8 banks
        pj = ctx.enter_context(tc.tile_pool(name="pj", bufs=2, space="PSUM"))
        ss = ctx.enter_context(tc.tile_pool(name="ss", bufs=2, space="PSUM"))
        yy = ctx.enter_context(tc.tile_pool(name="yy", bufs=1, space="PSUM"))

        cst = ctx.enter_context(tc.tile_pool(name="cst", bufs=1))
        xcp = ctx.enter_context(tc.tile_pool(name="xcp", bufs=3))
        ktp = ctx.enter_context(tc.tile_pool(name="ktp", bufs=2))
        vpp = ctx.enter_context(tc.tile_pool(name="vpp", bufs=2))
        qsp = ctx.enter_context(tc.tile_pool(name="qsp", bufs=4))
        qtp = ctx.enter_context(tc.tile_pool(name="qtp", bufs=8))
        tmp = ctx.enter_context(tc.tile_pool(name="tmp", bufs=4))
        ppp = ctx.enter_context(tc.tile_pool(name="ppp", bufs=4))
        ytu = ctx.enter_context(tc.tile_pool(name="ytu", bufs=2))
        ytp = ctx.enter_context(tc.tile_pool(name="ytp", bufs=4))
        osp = ctx.enter_context(tc.tile_pool(name="osp", bufs=4))
        ncp = ctx.enter_context(tc.tile_pool(name="ncp", bufs=4))

        # ---- constants ----
        XW = cst.tile([128, CT, 256], BF16, tag="XW")
        KW = cst.tile([128, CT, 64], BF16, tag="KW")
        VW = cst.tile([128, CT, 64], BF16, tag="VW")
        OW = cst.tile([128, 2, C], BF16, tag="OW")
        RQC = cst.tile([128, T], BF16, tag="RQC")
        RQS = cst.tile([128, T], BF16, tag="RQS")
        RKC = cst.tile([64, T], BF16, tag="RKC")
        RKS = cst.tile([64, T], BF16, tag="RKS")
        TRI = cst.tile([128, 2, 128], BF16, tag="TRI")

        PROJ_ROPES = {}
        XC = {}   # (b, ch) -> x chunk tile [128, CT, AC]
        KT = {}   # b -> [64, T]
        VP = {}   # b -> [128, KPB, 65]
        QT = {}   # (ch%2, h) -> [64, AC]
        YT = {}   # (b, cl) -> [128, T]

        copy_flip = [0]

        def stage_copy(dst, src):
            """PSUM->SBUF staging, alternating DVE / Act."""
            copy_flip[0] ^= 1
            if copy_flip[0]:
                nc.vector.tensor_copy(dst, src)
            else:
                nc.scalar.copy(dst, src)

        def emit_xdma(b, ch):
            t = xcp.tile([128, CT, AC], BF16, tag="XC", name=f"XC{b}_{ch}")
            nc.sync.dma_start(t[:, :, :], xTr[:, :, b * T + ch * AC:b * T + (ch + 1) * AC])
            XC[(b, ch)] = t

        def fillers_proj(b, ch):
            """Closures projecting chunk (b, ch): K, V, Q0, Q1."""
            xc = XC[(b, ch)]
            tcol = ch * AC
            res = []

            pk_box, pv_box = [], []
            ks_box, qs_box = [], {}

            def k_mm(c0):
                if c0 == 0:
                    pk_box.append(pj.tile([64, AC], F32, tag="pj", name=f"PK{b}_{ch}"))
                pk = pk_box[0]
                for c in range(c0, c0 + 4):
                    nc.tensor.matmul(pk[:], KW[:, c, :], xc[:, c, :],
                                     start=(c == 0), stop=(c == CT - 1))

            def k_copy():
                ks = qsp.tile([64, AC], BF16, tag="KS", name=f"KS{b}_{ch}")
                nc.vector.tensor_copy(ks[:], pk_box[0][:])
                ks_box.append(ks)

            def k_rope():
                ks = ks_box[0]
                kt = KT[b]
                t1 = tmp.tile([64, AC], BF16, tag="kt1")
                t2 = tmp.tile([64, AC], BF16, tag="kt2")
                nc.vector.tensor_mul(t1[:], ks[:], RKC[:, tcol:tcol + AC])
                nc.vector.tensor_mul(t2[0:32, :], ks[32:64, :], RKS[32:64, tcol:tcol + AC])
                nc.vector.tensor_mul(t2[32:64, :], ks[0:32, :], RKS[0:32, tcol:tcol + AC])
                nc.vector.tensor_add(kt[:, tcol:tcol + AC], t1[:], t2[:])

            def v_mm(tb):
                if tb == 0:
                    pv_box.append(pj.tile([128, 4, 64], F32, tag="pj", name=f"PV{b}_{ch}"))
                pv = pv_box[0]
                for c in range(CT):
                    nc.tensor.matmul(pv[:, tb, :],
                                     xc[:, c, tb * 128:(tb + 1) * 128], VW[:, c, :],
                                     start=(c == 0), stop=(c == CT - 1))

            def v_stage():
                nc.vector.tensor_copy(VP[b][:, ch * 4:(ch + 1) * 4, 0:64], pv_box[0][:, :, :])

            def q_mm(c0, hp, pq_box):
                if c0 == 0:
                    pq_box.append(pj.tile([128, AC], F32, tag="pj", name=f"PQ{b}_{ch}_{hp}"))
                pq = pq_box[0]
                for c in range(c0, c0 + 4):
                    nc.tensor.matmul(pq[:], XW[:, c, hp * 128:(hp + 1) * 128], xc[:, c, :],
                                     start=(c == 0), stop=(c == CT - 1))

            def q_copy(hp, pq_box):
                qs = qsp.tile([128, AC], BF16, tag="QS", name=f"QS{b}_{ch}_{hp}")
                nc.vector.tensor_copy(qs[:], pq_box[0][:])
                qs_box[hp] = qs

            def q_rope(hp):
                qs = qs_box[hp]
                t1 = tmp.tile([128, AC], BF16, tag="qt1")
                t2 = tmp.tile([128, AC], BF16, tag="qt2")
                nc.vector.tensor_mul(t1[:], qs[:], RQC[:, tcol:tcol + AC])
                for b0 in (0, 64):
                    nc.vector.tensor_mul(t2[b0:b0 + 32, :], qs[b0 + 32:b0 + 64, :],
                                         RQS[b0 + 32:b0 + 64, tcol:tcol + AC])
                    nc.vector.tensor_mul(t2[b0 + 32:b0 + 64, :], qs[b0:b0 + 32, :],
                                         RQS[b0:b0 + 32, tcol:tcol + AC])
                for hl in range(2):
                    h = 2 * hp + hl
                    qt = qtp.tile([64, AC], BF16, tag="QT", name=f"QT{b}_{ch}_{h}")
                    nc.vector.tensor_add(qt[:], t1[hl * 64:hl * 64 + 64, :],
                                         t2[hl * 64:hl * 64 + 64, :])
                    QT[(ch % 2, h)] = qt

            # psum->sbuf copies right behind each chain (frees pj bufs fast);
            # long DVE rope bursts deferred to the end
            for c0 in range(0, CT, 4):
                res.append((875, lambda c0=c0: k_mm(c0)))
            res.append((10, k_copy))
            for tb in range(4):
                res.append((430, lambda tb=tb: v_mm(tb)))
            res.append((10, v_stage))
            pq_boxes = [[], []]
            for hp in range(2):
                for c0 in range(0, CT, 4):
                    res.append((875, lambda c0=c0, hp=hp: q_mm(c0, hp, pq_boxes[hp])))
                res.append((10, lambda hp=hp: q_copy(hp, pq_boxes[hp])))
            ropes = [(150, k_rope), (150, lambda: q_rope(0)),
                     (150, lambda: q_rope(1))]
            return res, ropes

        TAIL = [False]

        def fillers_outproj(b, ch):
            """Closures for out projection of chunk (b, ch) (needs YT cols).

            Two co-columns per group share one [128,1024] staging tile and a
            single DMA — halves the serialized HWDGE holds (625ns each)."""
            res = []

            os_box = {}

            def po_group(tt, co2):
                trow = b * T + ch * AC + tt * 128
                if co2 == 0:
                    os_box[tt] = osp.tile([128, 2048], BF16, tag="OS", name=f"OS{b}_{ch}_{tt}")
                os_ = os_box[tt]
                for j in range(2):
                    co = 2 * co2 + j
                    po = pj.tile([128, 512], F32, tag="pj", name=f"PO{b}_{ch}_{tt}_{co}")
                    for cl in range(2):
                        nc.tensor.matmul(po[:], YT[(b, cl)][:, ch * 4 * 128 + tt * 128:ch * 4 * 128 + (tt + 1) * 128],
                                         OW[:, cl, co * 512:(co + 1) * 512],
                                         start=(cl == 0), stop=(cl == 1))
                    if j == 0:
                        nc.vector.tensor_copy(os_[:, co2 * 1024:co2 * 1024 + 512], po[:])
                    else:
                        nc.scalar.copy(os_[:, co2 * 1024 + 512:co2 * 1024 + 1024], po[:])
                if TAIL[0]:
                    nc.sync.dma_start(out[trow:trow + 128, co2 * 1024:(co2 + 1) * 1024],
                                      os_[:, co2 * 1024:(co2 + 1) * 1024])
                elif co2 == 1:
                    nc.sync.dma_start(out[trow:trow + 128, :], os_[:])

            for tt in range(4):
                for co2 in range(C // 1024):
                    res.append((880, lambda tt=tt, co2=co2: po_group(tt, co2)))
            return res

        projq = deque()   # (cost_ns, closure) — must drain before next chunk
        sideq = deque()   # out-proj groups — drain lazily
        qcost = [0]       # total cost queued

        def push(q, items):
            q.extend(items)
            qcost[0] += sum(c for c, _ in items)

        reserve = [0]

        def head_cost():
            if projq:
                return projq[0][0]
            if sideq and qcost[0] > reserve[0]:
                return sideq[0][0]
            return None

        def fill(budget):
            while budget > 0:
                if projq:
                    c, f = projq.popleft()
                elif sideq and qcost[0] > reserve[0]:
                    c, f = sideq.popleft()
                else:
                    return
                f()
                qcost[0] -= c
                budget -= c

        def flush_proj():
            while projq:
                c, f = projq.popleft()
                qcost[0] -= c
                f()

        def flush_all():
            flush_proj()
            while sideq:
                c, f = sideq.popleft()
                qcost[0] -= c
                f()

        def attention_chunk(b, ch, budget, last_chunk):
            """Both head-pair passes of chunk (b, ch) as one pipelined stream."""
            kt = KT[b]
            vp = VP[b]
            kis = [("d", l) for l in range(4)] + [("f", k) for k in range(ch * 4)]
            n = len(kis)
            ytuc = ytu.tile([65, 4, AC], BF16, tag="YTU", name=f"YTU{b}_{ch}")
            yps = {}
            pend = deque()

            def emit_av(e):
                p2, f0, pr, idx, ki_ = e
                for j in range(2):
                    nc.tensor.matmul(yps[pr][:, j, f0:512], vp[:, ki_, :], p2[:, j, f0:512],
                                     start=(idx == 0), stop=(idx == n - 1))
                if idx == n - 1:
                    if last_chunk and pr == 1:
                        normalize_direct(b, ch, yps[pr], pr)
                    else:
                        normalize(b, ch, ytuc, yps[pr], pr)

            for pr in range(2):
                for i, (kind, v) in enumerate(kis):
                    ki = ch * 4 + v if kind == "d" else v
                    f0 = v * 128 if kind == "d" else 0
                    if i == 0:
                        yps[pr] = yy.tile([65, 2, 512], F32, tag="yy", name=f"Y{b}_{ch}_{pr}")
                    s2 = ss.tile([128, 2, 512], F32, tag="ss", name=f"S{b}_{ch}_{pr}_{i}")
                    for j in range(2):
                        h = 2 * pr + j
                        nc.tensor.matmul(s2[:, j, f0:512], kt[:, ki * 128:(ki + 1) * 128],
                                         QT[(ch % 2, h)][:, f0:512], start=True, stop=True)
                    p2 = ppp.tile([128, 2, 512], BF16, tag="P2")
                    nc.scalar.activation(p2[:, :, f0:512], s2[:, :, f0:512], EXP)
                    if kind == "d":
                        nc.vector.tensor_mul(p2[:, :, f0:f0 + 128], p2[:, :, f0:f0 + 128], TRI[:, :, :])
                    pend.append((p2, f0, pr, i, ki))
                    if len(pend) > 2:
                        emit_av(pend.popleft())
                    fill(800)
            while pend:
                emit_av(pend.popleft())

        def normalize(b, ch, ytuc, yps, pr):
            """Normalize the two heads of pass pr into YT.

            Recips read the denominator rows straight from PSUM (ahead of the
            Y drain); the two multiplies run on different engines in parallel.
            """
            tcol = ch * AC
            nc.vector.tensor_copy(ytuc[:, 2 * pr:2 * pr + 2, :], yps[:, :, :])
            rcs = []
            for j in range(2):
                rc = ncp.tile([1, AC], F32, tag="RC")
                nc.vector.reciprocal(rc[0:1, :], ytuc[64:65, 2 * pr + j, :])
                rcs.append(rc)
            bcs = []
            for j in range(2):
                bc = ncp.tile([64, AC], F32, tag="BC")
                nc.gpsimd.partition_broadcast(bc[:], rcs[j][0:1, :])
                bcs.append(bc)
            for j in range(2):
                h = 2 * pr + j
                cl, r0 = h // 2, (h % 2) * 64
                nc.gpsimd.tensor_mul(YT[(b, cl)][r0:r0 + 64, tcol:tcol + AC],
                                     ytuc[0:64, h, :], bcs[j][:])

        def normalize_direct(b, ch, yps, pr):
            """Tail fast path: normalize straight from PSUM (no drain copy),
            DVE muls, h-pipelined recip/bcast."""
            tcol = ch * AC
            rcs, bcs = [], []
            for j in range(2):
                rc = ncp.tile([1, AC], F32, tag="RC")
                nc.vector.reciprocal(rc[0:1, :], yps[64:65, j, :])
                rcs.append(rc)
            for j in range(2):
                bc = ncp.tile([64, AC], F32, tag="BC")
                nc.gpsimd.partition_broadcast(bc[:], rcs[j][0:1, :])
                bcs.append(bc)
            for j in range(2):
                h = 2 * pr + j
                cl, r0 = h // 2, (h % 2) * 64
                nc.vector.tensor_mul(YT[(b, cl)][r0:r0 + 64, tcol:tcol + AC],
                                     yps[0:64, j, :], bcs[j][:])

        # ---- PE warmup: ramp the p-state while the first DMAs land ----
        WRM = cst.tile([128, 512], BF16, tag="WRM")
        nc.vector.memset(WRM[:], 0.0)
        pwarm = pj.tile([128, 512], F32, tag="pj", name="PWARM")
        for _ in range(18):
            nc.tensor.matmul(pwarm[:], WRM[:, 0:128], WRM[:], start=True, stop=True)

        # ---- preamble DMAs (need-ordered; x chunk 0 split so K proj can
        #      start as soon as its weights + first c-rows arrive) ----
        x00 = xcp.tile([128, CT, AC], BF16, tag="XC", name="XC0_0")
        XC[(0, 0)] = x00
        nc.sync.dma_start(KW[:, 0:8, :], wkr[:, 0:8, :])
        nc.sync.dma_start(x00[:, 0:4, :], xTr[:, 0:4, 0:AC])
        nc.sync.dma_start(KW[:, 8:CT, :], wkr[:, 8:CT, :])
        for q in range(1, 4):
            nc.sync.dma_start(x00[:, 4 * q:4 * (q + 1), :],
                              xTr[:, 4 * q:4 * (q + 1), 0:AC])
        nc.sync.dma_start(RKC[:], rkc[:])
        nc.sync.dma_start(RKS[:], rks[:])
        nc.sync.dma_start(VW[:, :, :], wvr[:, :, :])
        nc.sync.dma_start(XW[:, :, :], wqr[:, :, :])
        nc.sync.dma_start(RQC[:], rqc[:])
        nc.sync.dma_start(RQS[:], rqs[:])
        nc.sync.dma_start(TRI[:, :, :], tri2[:, :, :])
        nc.sync.dma_start(OW[:, :, :], wor[:, :, :])
        emit_xdma(0, 1)

        for b in range(B):
            KT[b] = ktp.tile([64, T], BF16, tag="KT", name=f"KT{b}")
            VP[b] = vpp.tile([128, KPB, 65], BF16, tag="VP", name=f"VP{b}")
            nc.vector.memset(VP[b][:, :, 64:65], 1.0)
            for cl in range(2):
                YT[(b, cl)] = ytp.tile([128, T], BF16, tag="YT", name=f"YT{b}_{cl}")

        ca, ra = fillers_proj(0, 0)
        for _, c in ca + ra:
            c()

        def succ(b, ch, k):
            t = b * NCH + ch + k
            return (t // NCH, t % NCH) if t < B * NCH else None

        pushed = set()
        for b in range(B):
            for ch in range(NCH):
                flush_proj()  # this chunk's proj must be complete
                nxt = succ(b, ch, 2)
                if nxt:
                    emit_xdma(*nxt)
                # queue fillers: chains for chunk+1 (if new) + its ropes,
                # then chains of chunk+2 (QT-parity-safe extra supply)
                nxt = succ(b, ch, 1)
                if nxt:
                    if nxt not in pushed:
                        ca, ra = fillers_proj(*nxt)
                        push(projq, ca)
                        pushed.add(nxt)
                        PROJ_ROPES[nxt] = ra
                    push(projq, PROJ_ROPES.pop(nxt))
                nxt = succ(b, ch, 2)
                if ch >= 2 and nxt and nxt not in pushed:
                    ca, ra = fillers_proj(*nxt)
                    push(projq, ca)
                    pushed.add(nxt)
                    PROJ_ROPES[nxt] = ra
                last = b == B - 1 and ch == NCH - 1
                reserve[0] = 4000 if last else 0
                attention_chunk(b, ch, 800, last)
                push(sideq, fillers_outproj(b, ch))
        TAIL[0] = True
        flush_all()

    nc.compile()
    return nc


def rope_tables(T, scale):
    inv = 1.0 / (ROPE_BASE ** (np.arange(0, D, 2, dtype=np.float32) / D))
    t = np.arange(T, dtype=np.float32)
    freqs = np.outer(t, inv)
    emb = np.concatenate([freqs, freqs], -1)
    cos = np.cos(emb).T.astype(np.float32) * scale
    sin = np.sin(emb).T.astype(np.float32) * scale
    sinX = np.empty((64, T), np.float32)
    sinX[0:32] = sin[32:64]
    sinX[32:64] = -sin[0:32]
    return np.ascontiguousarray(cos), np.ascontiguousarray(sinX)


def _pk(a, nblk):
    """[nblk*128, F] -> [128, nblk, F] contiguous bf16."""
    n, f = a.shape
    return np.ascontiguousarray(
        a.reshape(nblk, 128, f).transpose(1, 0, 2)).astype(BF16_NP)


def make_inputs(x, Wq, Wk, Wv, Wo):
    B, T, C = x.shape
    CT = C // 128
    xT = np.ascontiguousarray(x.reshape(B * T, C).T)
    qc, qs = rope_tables(T, 1.0 / np.sqrt(D).astype(np.float32))
    kc, ks = rope_tables(T, 1.0)
    tri = np.triu(np.ones((128, 128), np.float32))
    common = {
        "xTr": _pk(xT, CT),
        "rqc": np.concatenate([qc, qc], 0).astype(BF16_NP),
        "rqs": np.concatenate([qs, qs], 0).astype(BF16_NP),
        "rkc": kc.astype(BF16_NP),
        "rks": ks.astype(BF16_NP),
        "tri2": np.ascontiguousarray(
            np.stack([tri, tri], 1)).astype(BF16_NP),
    }
    in_maps = []
    for i in range(8):
        m = dict(common)
        m["wqr"] = _pk(np.ascontiguousarray(Wq[:, i * 256:(i + 1) * 256]), CT)
        m["wkr"] = _pk(np.ascontiguousarray(Wk[:, i * 64:(i + 1) * 64]), CT)
        m["wvr"] = _pk(np.ascontiguousarray(Wv[:, i * 64:(i + 1) * 64]), CT)
        m["wor"] = _pk(np.ascontiguousarray(Wo[i * 256:(i + 1) * 256, :]), 2)
        in_maps.append(m)
    return in_maps


_NC_CACHE = {}


def _get_nc(C, T, B):
    key = (C, T, B)
    if key not in _NC_CACHE:
        _NC_CACHE[key] = build_nc(C, T, B)
    return _NC_CACHE[key]


def run(x, Wq, Wk, Wv, Wo, trace=False):
    from concourse.bass_utils import run_bass_kernel_spmd

    B, T, C = x.shape
    nc = _get_nc(C, T, B)
    in_maps = make_inputs(x, Wq, Wk, Wv, Wo)
    for attempt in range(3):
        try:
            res = run_bass_kernel_spmd(nc, in_maps, list(range(8)), trace=trace)
        except (ImportError, ModuleNotFoundError):
            res = run_bass_kernel_spmd(nc, in_maps, list(range(8)), trace=False)
        acc = res.results[0]["out"].astype(np.float32)
        for i in range(1, 8):
            acc = acc + res.results[i]["out"].astype(np.float32)
        if np.isfinite(acc).all():
            break
    return acc.reshape(B, T, C), res


def kernel(x, Wq, Wk, Wv, Wo):
    out, _ = run(x, Wq, Wk, Wv, Wo, trace=False)
    return out
